# revision 99
# baseline (speedup 1.0000x reference)
"""Bahdanau additive-attention kernel for one TRN2 chip (8 NeuronCores).

Reference computation (per batch b):
    q      = dec[b] @ w2 + b2 + b1                      # [1, E]
    H      = enc[b] @ w1                                # [S, E]
    scores = tanh(H + q) @ v (+ bv, softmax-invariant)  # [S, 1]
    attn   = softmax(scores over S)
    out[b] = attn @ enc[b]                              # [E]

Sharding: pure data-parallel over batch. 32 batches / 8 cores = 4 per core.
No collectives. Weights replicated. The host passes enc twice: transposed
([b, e, s]) in fp8-e4m3 for the H matmul, and natural layout in bf16 for the
context reduction.

The dominant H matmul runs in fp8 (e4m3) with MatmulPerfMode.DoubleRow: each
PE instruction contracts TWO 128-row k-chunks (lhsT [128,2,M], rhs [128,2,N])
at fp8's double rate - 2x the bf16/fp32r matmul throughput. w1 is pre-scaled
by 64 on the host so its [-1/32, 1/32] entries land in e4m3's normal range;
the 1/64 descale is fused into the ScalarE tanh (tanh(psum/64 + q)).
Quantization puts the end-to-end relative error at ~1.1e-2 (gate: 2e-2);
the fp8 products accumulate exactly in fp32 PSUM so hardware matches the
host-side estimate.

Per-core dataflow (B=4, S=2048, E=1024), working H^T = w1^T @ enc^T so the
tanh bias (q) is a per-partition scalar fused into the ScalarE activation:

  per s-block of 512:
    encT [e-chunk, s]   <- one consolidated DMA from host-transposed fp8 enc
    H^T chunks          <- 16 DoubleRow PE matmuls (w1 stationary)
    tanh(+q, /64)       <- ScalarE, PSUM -> SBUF (bf16)
    [lagged 1 block]  scores[1, s] = v^T @ tanh as 8 PE matmuls (vT column
                      stationary, tanh moving, PSUM-accumulated)
                      exp on ScalarE (+running sums); attn weights to DRAM
                      and back transposed ([s%128, s/128] layout)
    [lagged 2 blocks] ctx[1, E] += attn^T @ enc chunks (DVE, bf16 enc)
  softmax normalization is deferred to one final scale by 1/sum(exp):
  scores are bounded (|tanh|<1, v fixed) so no max-subtraction is needed.

The one-block lag of the v/exp stage and two-block lag of the context stage
keep the PE stream dense. The q projection (dec @ w2, computed directly in
transposed [e-part, b] layout) is injected into the PE stream after the
third H group so the opening matmuls never wait on the 2MB w2 load; the
first three tanhs are deferred until q lands. For the last batch the ctx
chain drains at lag 1 and its final s-block contributes via rank-1 PE
matmuls accumulated straight into the output-reduction PSUM group, keeping
the serial DVE chain out of the kernel's drain tail.

HW notes learned the hard way (all deterministic, simulator-invisible):
  - the first DMA into an SBUF region reused from earlier-scope tiles, when
    queued near 4-byte-stride gather descriptors, lands with the low 12
    mantissa bits of each aligned word zeroed -> main pools are allocated
    before the setup pool and the first encT tile is prefetched before any
    q-side DMAs;
  - fp8-typed ExternalInput uploads can corrupt; fp8 bytes travel as uint8
    and the DRAM APs are bitcast to fp8 in-kernel;
  - SBUF->SBUF partition-scatter DMA corrupts -> the exp transpose goes
    through DRAM;
  - each dma_start costs ~0.4us of sync-queue dispatch -> multi-chunk
    loads are consolidated into single multi-dim DMAs.
"""

import os
import sys

sys.path.insert(0, "/opt/trn_rl_repo")

import numpy as np  # noqa: E402

import concourse.tile as tile  # noqa: E402
from concourse import bacc, mybir  # noqa: E402
from concourse.bass import ts  # noqa: E402
from concourse.bass_utils import run_bass_kernel_spmd  # noqa: E402

P = 128
N_CORES = 8
B_TOTAL = 32
B = B_TOTAL // N_CORES  # 4 batches per core
S = 2048
E = 1024
EC = E // P  # 8 chunks of the hidden dim
EC2 = EC // 2  # 4 double-chunks (DoubleRow pairs)
SB = 512  # s-block (matmul moving size)
NSB = S // SB  # 4 s-blocks per batch
SK = S // P  # 16 s-chunks of 128 per batch
KSB = SB // P  # 4 s-chunks per s-block

F32 = mybir.dt.float32
F32R = mybir.dt.float32r
BF16 = mybir.dt.bfloat16
F8 = mybir.dt.float8e4  # e4m3
U8 = mybir.dt.uint8  # fp8 bytes travel as uint8: the fp8-typed host->device
# upload path corrupts part of the array; same bytes as uint8 arrive intact

W1_SCALE = 64.0  # host multiplies w1 by this before fp8 quantization

SD = F32R  # storage dtype of the DVE-side dataflow (bitcast f32)
Act = mybir.ActivationFunctionType
DR = mybir.MatmulPerfMode.DoubleRow

# bisection switches (temporary): set to "f32r" to revert a piece to baseline
Q_DT = F32R if os.environ.get("ATTN_Q") == "f32r" else BF16
CENC_DT = F32R if os.environ.get("ATTN_CENC") == "f32r" else BF16
H_FP8 = os.environ.get("ATTN_H") != "f32r"
WARM_GROUPS = int(os.environ.get("ATTN_WARM", "0"))


def _f32(ap):
    return ap if ap.dtype is F32 else ap.bitcast(F32)


DEBUG = os.environ.get("ATTN_DEBUG") == "1"


def _build_body(nc, tc, ctx, enc, encT_d, dec, w1, b1, w2, b2, v, out, dbg):
    # ---------------- persistent constants ----------------
    const = ctx.enter_context(tc.tile_pool(name="const", bufs=1))
    dram = ctx.enter_context(tc.tile_pool(name="dram", bufs=2, space="DRAM"))

    qT = const.tile([P, EC, B], F32)  # [p, c, b] = q_full[b, c*128+p]
    ones_f = const.tile([P, 1], F32)
    ones_sd = const.tile([P, 1], SD, name="ones_sd")
    ones_b = const.tile([P, 1], BF16, name="ones_b")
    nc.vector.memset(ones_f[:], 1.0)
    nc.vector.tensor_copy(ones_sd[:], ones_f[:])
    nc.vector.memset(ones_b[:], 1.0)

    # ---------------- main pools ----------------
    # Created BEFORE the setup pool: the first encT DMA must not land in a
    # region previously touched by the setup tiles / the 4-byte-stride qT
    # gather DMAs -- on HW that combination deterministically truncated the
    # low mantissa bits of the first encT tile (reduced-precision DMA path).
    encT_pool = ctx.enter_context(tc.tile_pool(name="encT", bufs=3))
    cenc_pool = ctx.enter_context(tc.tile_pool(name="cenc", bufs=5))
    work = ctx.enter_context(tc.tile_pool(name="work", bufs=18))
    accp = ctx.enter_context(tc.tile_pool(name="accp", bufs=2))
    onep = ctx.enter_context(tc.tile_pool(name="onep", bufs=2))
    ps_h = ctx.enter_context(tc.tile_pool(name="ps_h", bufs=4, space="PSUM"))
    ps_s = ctx.enter_context(tc.tile_pool(name="ps_s", bufs=1, space="PSUM"))
    ps_c = ctx.enter_context(tc.tile_pool(name="ps_c", bufs=1, space="PSUM"))

    def encT_dma(b, sb, split=1):
        encT = encT_pool.tile([P, EC, SB], F8 if H_FP8 else SD, tag="encT")
        encT_ap = encT_d[:].bitcast(F8) if H_FP8 else encT_d[:]
        encT_r = encT_ap[b].rearrange("(c p) s -> p c s", p=P)
        g = EC // split
        for i in range(split):
            nc.sync.dma_start(
                encT[:, ts(i, g), :], encT_r[:, ts(i, g), ts(sb, SB)]
            )
        return encT

    # ---- setup (pools stay open: the deferred q issue uses them later) ----
    if True:
        setup = ctx.enter_context(tc.tile_pool(name="setup", bufs=1))
        setup_ps = ctx.enter_context(
            tc.tile_pool(name="setup_ps", bufs=1, space="PSUM")
        )
        w2_sb = setup.tile([P, EC, E], Q_DT)
        w2_r = w2[:].rearrange("(c p) e -> p c e", p=P)
        decT = setup.tile([P, EC, B], Q_DT)  # [p, c, b] = dec[b, 0, c*128+p]
        dec_r = dec[:][:, 0, :].rearrange("b (c p) -> p c b", p=P)
        for c in range(EC):
            nc.sync.dma_start(decT[:, c, :], dec_r[:, c, :])
        b12T = setup.tile([P, EC], F32)
        b1_sb = setup.tile([P, EC], F32)
        b2_sb = setup.tile([P, EC], F32)
        nc.sync.dma_start(b1_sb[:], b1[:].rearrange("(c p) -> p c", p=P))
        nc.sync.dma_start(b2_sb[:], b2[:].rearrange("(c p) -> p c", p=P))
        nc.vector.tensor_add(b12T[:], b1_sb[:], b2_sb[:])

        # w1 + the first encT tile stream FIRST: they gate the opening H
        # matmuls. w2 (which only gates q/tanh) streams after them.
        w1_sb = const.tile([P, EC, E], F8 if H_FP8 else SD)  # w1[c*128+p, e']
        w1_ap = w1[:].bitcast(F8) if H_FP8 else w1[:]
        w1_r = w1_ap.rearrange("(c p) e -> p c e", p=P)
        nc.sync.dma_start(w1_sb[:], w1_r[:])
        vT = const.tile([P, EC], SD)  # [p, c] = v[c*128+p, 0]
        nc.sync.dma_start(vT[:], v[:][:, 0].rearrange("(c p) -> p c", p=P))
        vT_b = const.tile([P, EC], BF16, name="vT_b")  # v-matmul stationary
        nc.vector.tensor_copy(vT_b[:], _f32(vT[:]))

        # prefetch the first s-block's encT ahead of the q/qT DMAs (see the
        # main-pool comment: ordering after them corrupts this tile on HW)
        encT_first = encT_dma(0, 0)
        nc.sync.dma_start(w2_sb[:], w2_r[:])

        # q is computed directly in [e'-partition, b] layout: stationary w2
        # chunk, moving decT columns -> PSUM [128, B]; bias add fuses b1+b2.
        # Deferred: issued into the PE stream AFTER the first H group so the
        # opening H matmuls don't wait behind the 2MB w2 load.
        def issue_q():
            for cp in range(EC):
                q_ps = setup_ps.tile([P, B], F32, tag="q_ps")
                for c in range(EC):
                    nc.tensor.matmul(
                        q_ps[:],
                        w2_sb[:, c, ts(cp, P)],
                        decT[:, c, :],
                        start=(c == 0),
                        stop=(c == EC - 1),
                    )
                nc.vector.tensor_scalar_add(
                    qT[:, cp, :], q_ps[:], b12T[:, cp : cp + 1]
                )
            if DEBUG:
                nc.sync.dma_start(dbg["qT"][:], qT[:])

    # PE warm-up: on HW the first fp8-DoubleRow window after the f32r/bf16
    # q matmuls computes corrupted PSUM (first-s-block-of-batch-0 signature;
    # later identical instructions are fine). Burn that window on dummy
    # DoubleRow groups whose results are discarded (a token column is DMA'd
    # out so the instructions aren't dead-code-eliminated).
    if H_FP8 and WARM_GROUPS > 0:
        warm_sb = const.tile([P, WARM_GROUPS], F32, name="warm_sb")
        for g in range(WARM_GROUPS):
            wps = ps_h.tile([P, SB], F32, tag="ph")
            for c2 in range(EC2):
                nc.tensor.matmul(
                    wps[:],
                    w1_sb[:, 2 * c2 : 2 * c2 + 2, 0:P],
                    w1_sb[:, 2 * c2 : 2 * c2 + 2, 0:SB],
                    start=(c2 == 0),
                    stop=(c2 == EC2 - 1),
                    perf_mode=DR,
                )
            nc.vector.tensor_copy(warm_sb[:, g : g + 1], wps[:, 0:1])
        warm_d = dram.tile([P, WARM_GROUPS], F32, tag="warm_d")
        nc.sync.dma_start(warm_d[:], warm_sb[:])

    # Work deferred so the PE never waits on ScalarE output or DMA
    # roundtrips: flushed one (v/exp) or two (ctx) s-blocks later.
    pending_v = []
    pending_ctx = []

    def flush_one(queue):
        if queue:
            queue.pop(0)()

    for b in range(B):
        a_dram = dram.tile([1, S], SD, tag="a_dram")
        sums = onep.tile([1, NSB], F32, tag="sums")
        expT = work.tile([P, SK], SD, tag="expT")  # [p, k] = exp[k*128+p]
        recip = onep.tile([1, 1], F32, tag="recip")
        cstate = {}  # running DVE accumulator for the context reduction

        for sb in range(NSB):
            if b == B - 1:
                # last batch: issue the previous block's v-stage FIRST so
                # its exp/roundtrip run ahead of this block's tanhs on the
                # Act queue - the final ctx chain (and the output reduction
                # behind it) then overlaps the last H blocks instead of
                # serializing into the drain tail
                flush_one(pending_v)
            # encT[p, c, j] = enc[b, sb*512+j, c*128+p], from host transpose.
            # The tile for block N+1 is DMA'd while block N computes (issue
            # pipelined one block ahead) so the PE never waits on it.
            if b == 0 and sb == 0:
                encT = encT_first
            else:
                encT = encT_prefetched  # noqa: F821 (set one iteration ago)
            nb, nsb = (b, sb + 1) if sb + 1 < NSB else (b + 1, 0)
            if nb < B:
                encT_prefetched = encT_dma(nb, nsb)
            # ---- main matmuls: H^T chunks via fp8 DoubleRow, tanh(+q) ----
            # Each DoubleRow instruction contracts e-chunks (2*c2, 2*c2+1):
            # lhsT [128, 2, 128] and rhs [128, 2, 512] pair along dim 1.
            def issue_tanh(ph, cp):
                th = work.tile([P, SB], BF16, tag="tanh")
                nc.scalar.activation(
                    th[:],
                    ph[:],
                    Act.Tanh,
                    bias=qT[:, cp, b : b + 1],
                    scale=(1.0 / W1_SCALE) if H_FP8 else 1.0,
                )
                return th

            ths = []
            pend_ph = []  # (b0, sb0): tanhs deferred until q lands in qT
            for cp in range(EC):
                ph = ps_h.tile([P, SB], F32, tag="ph")
                if H_FP8:
                    for c2 in range(EC2):
                        nc.tensor.matmul(
                            ph[:],
                            w1_sb[:, 2 * c2 : 2 * c2 + 2, ts(cp, P)],
                            encT[:, 2 * c2 : 2 * c2 + 2, :],
                            start=(c2 == 0),
                            stop=(c2 == EC2 - 1),
                            perf_mode=DR,
                        )
                else:
                    for c in range(EC):
                        nc.tensor.matmul(
                            ph[:],
                            w1_sb[:, c, ts(cp, P)],
                            encT[:, c, :],
                            start=(c == 0),
                            stop=(c == EC - 1),
                        )
                if b == 0 and sb == 0 and cp <= 2:
                    # hold the first tanhs: their qT bias is produced by the
                    # q matmuls injected after the third H group (so the
                    # opening H stream never waits on the w2 load)
                    pend_ph.append((cp, ph))
                    if cp == 2:
                        issue_q()
                        ths.extend(issue_tanh(p, c) for c, p in pend_ph)
                else:
                    ths.append(issue_tanh(ph, cp))

            if DEBUG and b == 0 and sb <= 1:
                nc.sync.dma_start(dbg["encT"][:][sb], encT[:])

            # prefetch the natural-layout bf16 enc chunks this block's
            # (2-block lagged) ctx reduction will need; issued after the
            # mains so they stay off the startup-critical DMA window
            enc_b = enc[:][b].rearrange("(k p) e -> p k e", p=P)
            cenc = cenc_pool.tile([P, KSB, E], CENC_DT, tag="cenc")
            nc.sync.dma_start(cenc[:], enc_b[:, ts(sb, KSB), :])
            cencs = [cenc[:, j, :] for j in range(KSB)]

            flush_one(pending_v)
            if len(pending_ctx) >= 2:
                flush_one(pending_ctx)
            if b == B - 1:
                # drain the ctx backlog to lag 1 during the last batch so
                # the remaining DVE chain segments overlap the final PE
                # blocks instead of serializing into the tail
                flush_one(pending_ctx)

            def make_v(
                b=b,
                sb=sb,
                ths=ths,
                sums=sums,
                expT=expT,
                recip=recip,
                a_dram=a_dram,
            ):
                def issue():
                    # scores[1, s] = sum_e v[e] * tanh[e, s] on the PE:
                    # vT chunk is a 1-column stationary (cheap ldweights),
                    # the tanh tiles stream as moving data; accumulate the
                    # 8 e-chunks in PSUM
                    pss = ps_s.tile([1, SB], F32, tag="pss", name="pss")
                    for cp in range(EC):
                        nc.tensor.matmul(
                            pss[:],
                            vT_b[:, cp : cp + 1],
                            ths[cp][:],
                            start=(cp == 0),
                            stop=(cp == EC - 1),
                        )
                    # exp + running sums (no max needed: |scores| <= 32)
                    exp_sb = onep.tile([1, SB], SD, tag="exp", name="exp_sb")
                    nc.scalar.activation(
                        exp_sb[:],
                        pss[:],
                        Act.Exp,
                        accum_out=sums[:, sb : sb + 1],
                    )
                    # transpose into expT[p, k] = exp[k*128+p] via a DRAM
                    # roundtrip (SBUF->SBUF partition-scatter DMA corrupts)
                    nc.sync.dma_start(a_dram[:, ts(sb, SB)], exp_sb[:])
                    nc.sync.dma_start(
                        expT[:, ts(sb, KSB)],
                        a_dram[:][0, ts(sb, SB)].rearrange(
                            "(k p) -> p k", p=P
                        ),
                    )
                    if DEBUG and sb == NSB - 1 and b <= 1:
                        nc.sync.dma_start(dbg["expT"][:][b], _f32(expT[:]))
                    if sb == NSB - 1:
                        # softmax denominator: must be issued AFTER the
                        # final sums write (Tile deps follow program order)
                        ssum = onep.tile([1, 1], F32, tag="ssum", name="ssum")
                        nc.vector.tensor_reduce(
                            ssum[:],
                            sums[:],
                            mybir.AxisListType.X,
                            mybir.AluOpType.add,
                        )
                        nc.vector.reciprocal(recip[:], ssum[:])

                return issue

            def make_ctx(
                b=b,
                sb=sb,
                cencs=cencs,
                expT=expT,
                cstate=cstate,
                recip=recip,
                last=(sb == NSB - 1),
            ):
                def issue():
                    # acc2[p, e] += enc[k*128+p, e] * attn[k*128+p]
                    # (VectorE); partition-sum via ones-matmul at the end.
                    # For the very last s-block of the last batch the serial
                    # DVE chain would be the kernel's drain tail - instead
                    # its 4 chunks go straight into the final PSUM group as
                    # rank-1 PE matmuls (attn column stationary).
                    pe_tail = b == B - 1 and last
                    if not pe_tail:
                        for j, k in enumerate(range(sb * KSB, (sb + 1) * KSB)):
                            nxt = accp.tile(
                                [P, E], SD, tag=f"cacc{k % 2}", name="cacc"
                            )
                            attn_k = _f32(expT[:, k : k + 1])
                            if k == 0:
                                nc.vector.tensor_scalar_mul(
                                    nxt[:], cencs[j], attn_k
                                )
                            else:
                                nc.vector.scalar_tensor_tensor(
                                    nxt[:],
                                    cencs[j],
                                    attn_k,
                                    cstate["acc"][:],
                                    mybir.AluOpType.mult,
                                    mybir.AluOpType.add,
                                )
                            cstate["acc"] = nxt
                    if last:
                        acc2 = cstate["acc"]
                        if pe_tail:
                            expT_b = onep.tile(
                                [P, KSB], BF16, tag="expT_b", name="expT_b"
                            )
                            nc.vector.tensor_copy(
                                expT_b[:], _f32(expT[:, ts(sb, KSB)])
                            )
                        for h in range(E // SB):
                            psc = ps_c.tile(
                                [1, SB], F32, tag=f"psc{h}", name="psc"
                            )
                            nc.tensor.matmul(
                                psc[:],
                                ones_sd[:],
                                acc2[:, ts(h, SB)],
                                start=True,
                                stop=not pe_tail,
                            )
                            if pe_tail:
                                for j in range(KSB):
                                    nc.tensor.matmul(
                                        psc[:],
                                        expT_b[:, j : j + 1],
                                        cencs[j][:, ts(h, SB)],
                                        start=False,
                                        stop=(j == KSB - 1),
                                    )
                            ctx_sb = onep.tile(
                                [1, SB], F32, tag="ctx", name="ctx_sb"
                            )
                            nc.scalar.activation(
                                ctx_sb[:], psc[:], Act.Copy, scale=recip[:]
                            )
                            nc.sync.dma_start(
                                out[:][b : b + 1, ts(h, SB)], ctx_sb[:]
                            )

                return issue

            pending_v.append(make_v())
            pending_ctx.append(make_ctx())

    while pending_v or pending_ctx:
        flush_one(pending_v)
        flush_one(pending_ctx)


def build_nc():
    nc = bacc.Bacc(
        "TRN2", target_bir_lowering=False, debug=False, num_devices=N_CORES
    )
    enc = nc.dram_tensor("encoder_outputs", [B, S, E], CENC_DT, kind="ExternalInput")
    encT_d = nc.dram_tensor(
        "encoder_outputs_t", [B, E, S], U8 if H_FP8 else SD, kind="ExternalInput"
    )
    dec = nc.dram_tensor("decoder_output", [B, 1, E], Q_DT, kind="ExternalInput")
    w1 = nc.dram_tensor("w1", [E, E], U8 if H_FP8 else SD, kind="ExternalInput")
    b1 = nc.dram_tensor("b1", [E], F32, kind="ExternalInput")
    w2 = nc.dram_tensor("w2", [E, E], Q_DT, kind="ExternalInput")
    b2 = nc.dram_tensor("b2", [E], F32, kind="ExternalInput")
    v = nc.dram_tensor("v", [E, 1], SD, kind="ExternalInput")
    out = nc.dram_tensor("out", [B, E], F32, kind="ExternalOutput")
    dbg = {}
    if DEBUG:
        dbg["qT"] = nc.dram_tensor("dbg_qT", [P, EC, B], F32, kind="ExternalOutput")
        dbg["th"] = nc.dram_tensor("dbg_th", [4, P, SB], F32, kind="ExternalOutput")
        dbg["ph"] = nc.dram_tensor("dbg_ph", [P, SB], F32, kind="ExternalOutput")
        dbg["expT"] = nc.dram_tensor("dbg_expT", [2, P, SK], F32, kind="ExternalOutput")
        dbg["encT"] = nc.dram_tensor(
            "dbg_encT", [2, P, EC, SB], F8 if H_FP8 else SD,
            kind="ExternalOutput"
        )

    from contextlib import ExitStack

    with tile.TileContext(nc) as tc:
        with ExitStack() as ctx:
            _build_body(nc, tc, ctx, enc, encT_d, dec, w1, b1, w2, b2, v, out, dbg)
    nc.compile()
    return nc


_NC_CACHE = None


def _get_nc():
    global _NC_CACHE
    if _NC_CACHE is None:
        _NC_CACHE = build_nc()
    return _NC_CACHE


def make_in_maps(inputs):
    """Host-side prep: shard over batch, quantize (fp8 transposed enc for the
    H matmul, bf16 natural enc for the context stage, fp8 w1 scaled by 64)."""
    f32 = np.float32
    q_np = mybir.dt.np(Q_DT)
    cenc_np = mybir.dt.np(CENC_DT)
    h_np = mybir.dt.np(F8) if H_FP8 else f32
    enc_all = np.asarray(inputs["encoder_outputs"], dtype=f32)
    enc_bf16 = np.ascontiguousarray(enc_all.astype(cenc_np))
    encT_f8 = np.ascontiguousarray(
        enc_all.astype(h_np).transpose(0, 2, 1)
    )
    dec_bf16 = np.asarray(inputs["decoder_output"], dtype=f32).astype(q_np)
    w1_f32 = np.asarray(inputs["w1"], dtype=f32)
    w1_f8 = (w1_f32 * f32(W1_SCALE)).astype(h_np) if H_FP8 else w1_f32
    if H_FP8:
        encT_f8 = encT_f8.view(np.uint8)
        w1_f8 = w1_f8.view(np.uint8)
    w2_bf16 = np.asarray(inputs["w2"], dtype=f32).astype(q_np)
    in_maps = []
    for i in range(N_CORES):
        sl = slice(i * B, (i + 1) * B)
        in_maps.append(
            {
                "encoder_outputs": np.ascontiguousarray(enc_bf16[sl]),
                "encoder_outputs_t": encT_f8[sl],
                "decoder_output": np.ascontiguousarray(dec_bf16[sl]),
                "w1": w1_f8,
                "b1": np.ascontiguousarray(inputs["b1"], dtype=f32),
                "w2": w2_bf16,
                "b2": np.ascontiguousarray(inputs["b2"], dtype=f32),
                "v": np.ascontiguousarray(inputs["v"], dtype=f32),
            }
        )
    return in_maps


def run(inputs, trace=False):
    """Run on hardware. Returns (output [32, 1024] f32, exec_time_ns or None)."""
    nc = _get_nc()
    in_maps = make_in_maps(inputs)
    res = run_bass_kernel_spmd(
        nc, in_maps, core_ids=list(range(N_CORES)), trace=trace
    )
    out = np.concatenate([np.asarray(r["out"]) for r in res.results], axis=0)
    return out, res.exec_time_ns


def kernel(**inputs):
    out, _ = run(inputs)
    return out


# revision 100
# speedup vs baseline: 1.1556x; 1.1556x over previous
"""Bahdanau additive-attention kernel for one TRN2 chip (8 NeuronCores).

Reference computation (per batch b):
    q      = dec[b] @ w2 + b2 + b1                      # [1, E]
    H      = enc[b] @ w1                                # [S, E]
    scores = tanh(H + q) @ v (+ bv, softmax-invariant)  # [S, 1]
    attn   = softmax(scores over S)
    out[b] = attn @ enc[b]                              # [E]

Sharding: pure data-parallel over batch. 32 batches / 8 cores = 4 per core.
No collectives. Weights replicated. The host passes enc twice: transposed
([b, e, s]) in fp8-e4m3 for the H matmul, and natural layout in bf16 for the
context reduction.

The dominant H matmul runs in fp8 (e4m3) with MatmulPerfMode.DoubleRow: each
PE instruction contracts TWO 128-row k-chunks (lhsT [128,2,M], rhs [128,2,N])
at fp8's double rate - 2x the bf16/fp32r matmul throughput. w1 is pre-scaled
by 64 on the host so its [-1/32, 1/32] entries land in e4m3's normal range;
the 1/64 descale is fused into the ScalarE tanh (tanh(psum/64 + q)).
Quantization puts the end-to-end relative error at ~1.1e-2 (gate: 2e-2);
the fp8 products accumulate exactly in fp32 PSUM so hardware matches the
host-side estimate.

Per-core dataflow (B=4, S=2048, E=1024), working H^T = w1^T @ enc^T so the
tanh bias (q) is a per-partition scalar fused into the ScalarE activation:

  per s-block of 512:
    encT [e-chunk, s]   <- one consolidated DMA from host-transposed fp8 enc
    H^T chunks          <- 16 DoubleRow PE matmuls (w1 stationary)
    tanh(+q, /64)       <- ScalarE, PSUM -> SBUF (bf16)
    [lagged 1 block]  scores[1, s] = v^T @ tanh as 8 PE matmuls (vT column
                      stationary, tanh moving, PSUM-accumulated)
                      exp on ScalarE (+running sums); attn weights to DRAM
                      and back transposed ([s%128, s/128] layout)
    [lagged 2 blocks] ctx[1, E] += attn^T @ enc chunks (DVE, bf16 enc)
  softmax normalization is deferred to one final scale by 1/sum(exp):
  scores are bounded (|tanh|<1, v fixed) so no max-subtraction is needed.

The one-block lag of the v/exp stage and two-block lag of the context stage
keep the PE stream dense. The q projection (dec @ w2, computed directly in
transposed [e-part, b] layout) is injected into the PE stream after the
third H group so the opening matmuls never wait on the 2MB w2 load; the
first three tanhs are deferred until q lands. For the last batch the ctx
chain drains at lag 1 and its final s-block contributes via rank-1 PE
matmuls accumulated straight into the output-reduction PSUM group, keeping
the serial DVE chain out of the kernel's drain tail.

HW notes learned the hard way (all deterministic, simulator-invisible):
  - the first DMA into an SBUF region reused from earlier-scope tiles, when
    queued near 4-byte-stride gather descriptors, lands with the low 12
    mantissa bits of each aligned word zeroed -> main pools are allocated
    before the setup pool and the first encT tile is prefetched before any
    q-side DMAs;
  - fp8-typed ExternalInput uploads can corrupt; fp8 bytes travel as uint8
    and the DRAM APs are bitcast to fp8 in-kernel;
  - SBUF->SBUF partition-scatter DMA corrupts -> the exp transpose goes
    through DRAM;
  - each dma_start costs ~0.4us of sync-queue dispatch -> multi-chunk
    loads are consolidated into single multi-dim DMAs.
"""

import os
import sys

sys.path.insert(0, "/opt/trn_rl_repo")

import numpy as np  # noqa: E402

import concourse.tile as tile  # noqa: E402
from concourse import bacc, mybir  # noqa: E402
from concourse.bass import ts  # noqa: E402
from concourse.bass_utils import run_bass_kernel_spmd  # noqa: E402

P = 128
N_CORES = 8
B_TOTAL = 32
B = B_TOTAL // N_CORES  # 4 batches per core
S = 2048
E = 1024
EC = E // P  # 8 chunks of the hidden dim
EC2 = EC // 2  # 4 double-chunks (DoubleRow pairs)
SB = 512  # s-block (matmul moving size)
NSB = S // SB  # 4 s-blocks per batch
SK = S // P  # 16 s-chunks of 128 per batch
KSB = SB // P  # 4 s-chunks per s-block

F32 = mybir.dt.float32
F32R = mybir.dt.float32r
BF16 = mybir.dt.bfloat16
F8 = mybir.dt.float8e4  # e4m3
U8 = mybir.dt.uint8  # fp8 bytes travel as uint8: the fp8-typed host->device
# upload path corrupts part of the array; same bytes as uint8 arrive intact

W1_SCALE = 64.0  # host multiplies w1 by this before fp8 quantization

SD = F32R  # storage dtype of the DVE-side dataflow (bitcast f32)
Act = mybir.ActivationFunctionType
DR = mybir.MatmulPerfMode.DoubleRow

# bisection switches (temporary): set to "f32r" to revert a piece to baseline
Q_DT = F32R if os.environ.get("ATTN_Q") == "f32r" else BF16
CENC_DT = F32R if os.environ.get("ATTN_CENC") == "f32r" else BF16
H_FP8 = os.environ.get("ATTN_H") != "f32r"
WARM_GROUPS = int(os.environ.get("ATTN_WARM", "0"))


def _f32(ap):
    return ap if ap.dtype is F32 else ap.bitcast(F32)


DEBUG = os.environ.get("ATTN_DEBUG") == "1"


def _build_body(nc, tc, ctx, enc, encT_d, dec, w1, b1, w2, b2, v, out, dbg):
    # ---------------- persistent constants ----------------
    const = ctx.enter_context(tc.tile_pool(name="const", bufs=1))
    dram = ctx.enter_context(tc.tile_pool(name="dram", bufs=2, space="DRAM"))

    qT = const.tile([P, EC, B], F32)  # [p, c, b] = q_full[b, c*128+p]
    ones_f = const.tile([P, 1], F32)
    ones_sd = const.tile([P, 1], SD, name="ones_sd")
    ones_b = const.tile([P, 1], BF16, name="ones_b")
    nc.vector.memset(ones_f[:], 1.0)
    nc.vector.tensor_copy(ones_sd[:], ones_f[:])
    nc.vector.memset(ones_b[:], 1.0)

    # ---------------- main pools ----------------
    # Created BEFORE the setup pool: the first encT DMA must not land in a
    # region previously touched by the setup tiles / the 4-byte-stride qT
    # gather DMAs -- on HW that combination deterministically truncated the
    # low mantissa bits of the first encT tile (reduced-precision DMA path).
    encT_pool = ctx.enter_context(tc.tile_pool(name="encT", bufs=3))
    cenc_pool = ctx.enter_context(tc.tile_pool(name="cenc", bufs=5))
    work = ctx.enter_context(tc.tile_pool(name="work", bufs=18))
    accp = ctx.enter_context(tc.tile_pool(name="accp", bufs=2))
    onep = ctx.enter_context(tc.tile_pool(name="onep", bufs=2))
    ps_h = ctx.enter_context(tc.tile_pool(name="ps_h", bufs=4, space="PSUM"))
    ps_s = ctx.enter_context(tc.tile_pool(name="ps_s", bufs=1, space="PSUM"))
    ps_c = ctx.enter_context(tc.tile_pool(name="ps_c", bufs=1, space="PSUM"))

    def encT_dma(b, sb, split=1):
        encT = encT_pool.tile([P, EC, SB], F8 if H_FP8 else SD, tag="encT")
        encT_ap = encT_d[:].bitcast(F8) if H_FP8 else encT_d[:]
        encT_r = encT_ap[b].rearrange("(c p) s -> p c s", p=P)
        g = EC // split
        for i in range(split):
            nc.sync.dma_start(
                encT[:, ts(i, g), :], encT_r[:, ts(i, g), ts(sb, SB)]
            )
        return encT

    # ---- setup (pools stay open: the deferred q issue uses them later) ----
    if True:
        setup = ctx.enter_context(tc.tile_pool(name="setup", bufs=1))
        setup_ps = ctx.enter_context(
            tc.tile_pool(name="setup_ps", bufs=1, space="PSUM")
        )
        w2_sb = setup.tile([P, EC, E], Q_DT)
        w2_r = w2[:].rearrange("(c p) e -> p c e", p=P)
        decT = setup.tile([P, EC, B], Q_DT)  # [p, c, b] = dec[b, 0, c*128+p]
        dec_r = dec[:][:, 0, :].rearrange("b (c p) -> p c b", p=P)
        for c in range(EC):
            nc.sync.dma_start(decT[:, c, :], dec_r[:, c, :])
        b12T = setup.tile([P, EC], F32)
        b1_sb = setup.tile([P, EC], F32)
        b2_sb = setup.tile([P, EC], F32)
        nc.sync.dma_start(b1_sb[:], b1[:].rearrange("(c p) -> p c", p=P))
        nc.sync.dma_start(b2_sb[:], b2[:].rearrange("(c p) -> p c", p=P))
        nc.vector.tensor_add(b12T[:], b1_sb[:], b2_sb[:])

        # w1 + the first encT tile stream FIRST: they gate the opening H
        # matmuls. w2 (which only gates q/tanh) streams after them.
        w1_sb = const.tile([P, EC, E], F8 if H_FP8 else SD)  # w1[c*128+p, e']
        w1_ap = w1[:].bitcast(F8) if H_FP8 else w1[:]
        w1_r = w1_ap.rearrange("(c p) e -> p c e", p=P)
        nc.sync.dma_start(w1_sb[:], w1_r[:])
        vT = const.tile([P, EC], SD)  # [p, c] = v[c*128+p, 0]
        nc.sync.dma_start(vT[:], v[:][:, 0].rearrange("(c p) -> p c", p=P))
        vT_b = const.tile([P, EC], BF16, name="vT_b")  # v-matmul stationary
        nc.vector.tensor_copy(vT_b[:], _f32(vT[:]))

        # prefetch the first s-block's encT ahead of the q/qT DMAs (see the
        # main-pool comment: ordering after them corrupts this tile on HW)
        encT_first = encT_dma(0, 0)
        nc.sync.dma_start(w2_sb[:], w2_r[:])

        # q is computed directly in [e'-partition, b] layout: stationary w2
        # chunk, moving decT columns -> PSUM [128, B]; bias add fuses b1+b2.
        # Deferred: issued into the PE stream AFTER the first H group so the
        # opening H matmuls don't wait behind the 2MB w2 load.
        def issue_q():
            for cp in range(EC):
                q_ps = setup_ps.tile([P, B], F32, tag="q_ps")
                for c in range(EC):
                    nc.tensor.matmul(
                        q_ps[:],
                        w2_sb[:, c, ts(cp, P)],
                        decT[:, c, :],
                        start=(c == 0),
                        stop=(c == EC - 1),
                    )
                nc.vector.tensor_scalar_add(
                    qT[:, cp, :], q_ps[:], b12T[:, cp : cp + 1]
                )
            if DEBUG:
                nc.sync.dma_start(dbg["qT"][:], qT[:])

    # PE warm-up: on HW the first fp8-DoubleRow window after the f32r/bf16
    # q matmuls computes corrupted PSUM (first-s-block-of-batch-0 signature;
    # later identical instructions are fine). Burn that window on dummy
    # DoubleRow groups whose results are discarded (a token column is DMA'd
    # out so the instructions aren't dead-code-eliminated).
    if H_FP8 and WARM_GROUPS > 0:
        warm_sb = const.tile([P, WARM_GROUPS], F32, name="warm_sb")
        for g in range(WARM_GROUPS):
            wps = ps_h.tile([P, SB], F32, tag="ph")
            for c2 in range(EC2):
                nc.tensor.matmul(
                    wps[:],
                    w1_sb[:, 2 * c2 : 2 * c2 + 2, 0:P],
                    w1_sb[:, 2 * c2 : 2 * c2 + 2, 0:SB],
                    start=(c2 == 0),
                    stop=(c2 == EC2 - 1),
                    perf_mode=DR,
                )
            nc.vector.tensor_copy(warm_sb[:, g : g + 1], wps[:, 0:1])
        warm_d = dram.tile([P, WARM_GROUPS], F32, tag="warm_d")
        nc.sync.dma_start(warm_d[:], warm_sb[:])

    # Work deferred so the PE never waits on ScalarE output or DMA
    # roundtrips: flushed one (v/exp) or two (ctx) s-blocks later.
    pending_v = []
    pending_ctx = []

    def flush_one(queue):
        if queue:
            queue.pop(0)()

    for b in range(B):
        a_dram = dram.tile([1, S], SD, tag="a_dram")
        sums = onep.tile([1, NSB], F32, tag="sums")
        expT = work.tile([P, SK], SD, tag="expT")  # [p, k] = exp[k*128+p]
        recip = onep.tile([1, 1], F32, tag="recip")
        cstate = {}  # running DVE accumulator for the context reduction

        for sb in range(NSB):
            # encT[p, c, j] = enc[b, sb*512+j, c*128+p], from host transpose.
            # The tile for block N+1 is DMA'd while block N computes (issue
            # pipelined one block ahead) so the PE never waits on it.
            if b == 0 and sb == 0:
                encT = encT_first
            else:
                encT = encT_prefetched  # noqa: F821 (set one iteration ago)
            nb, nsb = (b, sb + 1) if sb + 1 < NSB else (b + 1, 0)
            if nb < B:
                encT_prefetched = encT_dma(nb, nsb)
            # ---- main matmuls: H^T chunks via fp8 DoubleRow, tanh(+q) ----
            # Each DoubleRow instruction contracts e-chunks (2*c2, 2*c2+1):
            # lhsT [128, 2, 128] and rhs [128, 2, 512] pair along dim 1.
            def issue_tanh(ph, cp):
                th = work.tile([P, SB], BF16, tag="tanh")
                nc.scalar.activation(
                    th[:],
                    ph[:],
                    Act.Tanh,
                    bias=qT[:, cp, b : b + 1],
                    scale=(1.0 / W1_SCALE) if H_FP8 else 1.0,
                )
                return th

            ths = []
            pend_ph = []  # (b0, sb0): tanhs deferred until q lands in qT
            for cp in range(EC):
                ph = ps_h.tile([P, SB], F32, tag="ph")
                if H_FP8:
                    for c2 in range(EC2):
                        nc.tensor.matmul(
                            ph[:],
                            w1_sb[:, 2 * c2 : 2 * c2 + 2, ts(cp, P)],
                            encT[:, 2 * c2 : 2 * c2 + 2, :],
                            start=(c2 == 0),
                            stop=(c2 == EC2 - 1),
                            perf_mode=DR,
                        )
                else:
                    for c in range(EC):
                        nc.tensor.matmul(
                            ph[:],
                            w1_sb[:, c, ts(cp, P)],
                            encT[:, c, :],
                            start=(c == 0),
                            stop=(c == EC - 1),
                        )
                if b == 0 and sb == 0 and cp <= 2:
                    # hold the first tanhs: their qT bias is produced by the
                    # q matmuls injected after the third H group (so the
                    # opening H stream never waits on the w2 load)
                    pend_ph.append((cp, ph))
                    if cp == 2:
                        issue_q()
                        ths.extend(issue_tanh(p, c) for c, p in pend_ph)
                else:
                    ths.append(issue_tanh(ph, cp))

            if DEBUG and b == 0 and sb <= 1:
                nc.sync.dma_start(dbg["encT"][:][sb], encT[:])

            # prefetch the natural-layout bf16 enc chunks this block's
            # (2-block lagged) ctx reduction will need; issued after the
            # mains so they stay off the startup-critical DMA window
            enc_b = enc[:][b].rearrange("(k p) e -> p k e", p=P)
            cenc = cenc_pool.tile([P, KSB, E], CENC_DT, tag="cenc")
            nc.sync.dma_start(cenc[:], enc_b[:, ts(sb, KSB), :])
            cencs = [cenc[:, j, :] for j in range(KSB)]

            flush_one(pending_v)
            if len(pending_ctx) >= 2:
                flush_one(pending_ctx)
            if b == B - 1:
                # drain the ctx backlog to lag 1 during the last batch so
                # the remaining DVE chain segments overlap the final PE
                # blocks instead of serializing into the tail
                flush_one(pending_ctx)

            def make_v(
                b=b,
                sb=sb,
                ths=ths,
                sums=sums,
                expT=expT,
                recip=recip,
                a_dram=a_dram,
            ):
                def issue():
                    # scores[1, s] = sum_e v[e] * tanh[e, s] on the PE:
                    # vT chunk is a 1-column stationary (cheap ldweights),
                    # the tanh tiles stream as moving data; accumulate the
                    # 8 e-chunks in PSUM
                    pss = ps_s.tile([1, SB], F32, tag="pss", name="pss")
                    for cp in range(EC):
                        nc.tensor.matmul(
                            pss[:],
                            vT_b[:, cp : cp + 1],
                            ths[cp][:],
                            start=(cp == 0),
                            stop=(cp == EC - 1),
                        )
                    # exp + running sums (no max needed: |scores| <= 32)
                    exp_sb = onep.tile([1, SB], SD, tag="exp", name="exp_sb")
                    nc.scalar.activation(
                        exp_sb[:],
                        pss[:],
                        Act.Exp,
                        accum_out=sums[:, sb : sb + 1],
                    )
                    # transpose into expT[p, k] = exp[k*128+p] via a DRAM
                    # roundtrip (SBUF->SBUF partition-scatter DMA corrupts)
                    nc.sync.dma_start(a_dram[:, ts(sb, SB)], exp_sb[:])
                    nc.sync.dma_start(
                        expT[:, ts(sb, KSB)],
                        a_dram[:][0, ts(sb, SB)].rearrange(
                            "(k p) -> p k", p=P
                        ),
                    )
                    if DEBUG and sb == NSB - 1 and b <= 1:
                        nc.sync.dma_start(dbg["expT"][:][b], _f32(expT[:]))
                    if sb == NSB - 1:
                        # softmax denominator: must be issued AFTER the
                        # final sums write (Tile deps follow program order)
                        ssum = onep.tile([1, 1], F32, tag="ssum", name="ssum")
                        nc.vector.tensor_reduce(
                            ssum[:],
                            sums[:],
                            mybir.AxisListType.X,
                            mybir.AluOpType.add,
                        )
                        nc.vector.reciprocal(recip[:], ssum[:])

                return issue

            def make_ctx(
                b=b,
                sb=sb,
                cencs=cencs,
                expT=expT,
                cstate=cstate,
                recip=recip,
                last=(sb == NSB - 1),
            ):
                def issue():
                    # acc2[p, e] += enc[k*128+p, e] * attn[k*128+p]
                    # (VectorE); partition-sum via ones-matmul at the end.
                    # For the very last s-block of the last batch the serial
                    # DVE chain would be the kernel's drain tail - instead
                    # its 4 chunks go straight into the final PSUM group as
                    # rank-1 PE matmuls (attn column stationary).
                    pe_tail = b == B - 1 and last
                    if not pe_tail:
                        for j, k in enumerate(range(sb * KSB, (sb + 1) * KSB)):
                            nxt = accp.tile(
                                [P, E], SD, tag=f"cacc{k % 2}", name="cacc"
                            )
                            attn_k = _f32(expT[:, k : k + 1])
                            if k == 0:
                                nc.vector.tensor_scalar_mul(
                                    nxt[:], cencs[j], attn_k
                                )
                            else:
                                nc.vector.scalar_tensor_tensor(
                                    nxt[:],
                                    cencs[j],
                                    attn_k,
                                    cstate["acc"][:],
                                    mybir.AluOpType.mult,
                                    mybir.AluOpType.add,
                                )
                            cstate["acc"] = nxt
                    if last:
                        acc2 = cstate["acc"]
                        if pe_tail:
                            expT_b = onep.tile(
                                [P, KSB], BF16, tag="expT_b", name="expT_b"
                            )
                            nc.vector.tensor_copy(
                                expT_b[:], _f32(expT[:, ts(sb, KSB)])
                            )
                        for h in range(E // SB):
                            psc = ps_c.tile(
                                [1, SB], F32, tag=f"psc{h}", name="psc"
                            )
                            nc.tensor.matmul(
                                psc[:],
                                ones_sd[:],
                                acc2[:, ts(h, SB)],
                                start=True,
                                stop=not pe_tail,
                            )
                            if pe_tail:
                                for j in range(KSB):
                                    nc.tensor.matmul(
                                        psc[:],
                                        expT_b[:, j : j + 1],
                                        cencs[j][:, ts(h, SB)],
                                        start=False,
                                        stop=(j == KSB - 1),
                                    )
                            ctx_sb = onep.tile(
                                [1, SB], F32, tag="ctx", name="ctx_sb"
                            )
                            nc.scalar.activation(
                                ctx_sb[:], psc[:], Act.Copy, scale=recip[:]
                            )
                            nc.sync.dma_start(
                                out[:][b : b + 1, ts(h, SB)], ctx_sb[:]
                            )

                return issue

            pending_v.append(make_v())
            pending_ctx.append(make_ctx())

    while pending_v or pending_ctx:
        flush_one(pending_v)
        flush_one(pending_ctx)


def build_nc():
    nc = bacc.Bacc(
        "TRN2", target_bir_lowering=False, debug=False, num_devices=N_CORES
    )
    enc = nc.dram_tensor("encoder_outputs", [B, S, E], CENC_DT, kind="ExternalInput")
    encT_d = nc.dram_tensor(
        "encoder_outputs_t", [B, E, S], U8 if H_FP8 else SD, kind="ExternalInput"
    )
    dec = nc.dram_tensor("decoder_output", [B, 1, E], Q_DT, kind="ExternalInput")
    w1 = nc.dram_tensor("w1", [E, E], U8 if H_FP8 else SD, kind="ExternalInput")
    b1 = nc.dram_tensor("b1", [E], F32, kind="ExternalInput")
    w2 = nc.dram_tensor("w2", [E, E], Q_DT, kind="ExternalInput")
    b2 = nc.dram_tensor("b2", [E], F32, kind="ExternalInput")
    v = nc.dram_tensor("v", [E, 1], SD, kind="ExternalInput")
    out = nc.dram_tensor("out", [B, E], F32, kind="ExternalOutput")
    dbg = {}
    if DEBUG:
        dbg["qT"] = nc.dram_tensor("dbg_qT", [P, EC, B], F32, kind="ExternalOutput")
        dbg["th"] = nc.dram_tensor("dbg_th", [4, P, SB], F32, kind="ExternalOutput")
        dbg["ph"] = nc.dram_tensor("dbg_ph", [P, SB], F32, kind="ExternalOutput")
        dbg["expT"] = nc.dram_tensor("dbg_expT", [2, P, SK], F32, kind="ExternalOutput")
        dbg["encT"] = nc.dram_tensor(
            "dbg_encT", [2, P, EC, SB], F8 if H_FP8 else SD,
            kind="ExternalOutput"
        )

    from contextlib import ExitStack

    with tile.TileContext(nc) as tc:
        with ExitStack() as ctx:
            _build_body(nc, tc, ctx, enc, encT_d, dec, w1, b1, w2, b2, v, out, dbg)
    nc.compile()
    return nc


_NC_CACHE = None


def _get_nc():
    global _NC_CACHE
    if _NC_CACHE is None:
        _NC_CACHE = build_nc()
    return _NC_CACHE


def make_in_maps(inputs):
    """Host-side prep: shard over batch, quantize (fp8 transposed enc for the
    H matmul, bf16 natural enc for the context stage, fp8 w1 scaled by 64)."""
    f32 = np.float32
    q_np = mybir.dt.np(Q_DT)
    cenc_np = mybir.dt.np(CENC_DT)
    h_np = mybir.dt.np(F8) if H_FP8 else f32
    enc_all = np.asarray(inputs["encoder_outputs"], dtype=f32)
    enc_bf16 = np.ascontiguousarray(enc_all.astype(cenc_np))
    encT_f8 = np.ascontiguousarray(
        enc_all.astype(h_np).transpose(0, 2, 1)
    )
    dec_bf16 = np.asarray(inputs["decoder_output"], dtype=f32).astype(q_np)
    w1_f32 = np.asarray(inputs["w1"], dtype=f32)
    w1_f8 = (w1_f32 * f32(W1_SCALE)).astype(h_np) if H_FP8 else w1_f32
    if H_FP8:
        encT_f8 = encT_f8.view(np.uint8)
        w1_f8 = w1_f8.view(np.uint8)
    w2_bf16 = np.asarray(inputs["w2"], dtype=f32).astype(q_np)
    in_maps = []
    for i in range(N_CORES):
        sl = slice(i * B, (i + 1) * B)
        in_maps.append(
            {
                "encoder_outputs": np.ascontiguousarray(enc_bf16[sl]),
                "encoder_outputs_t": encT_f8[sl],
                "decoder_output": np.ascontiguousarray(dec_bf16[sl]),
                "w1": w1_f8,
                "b1": np.ascontiguousarray(inputs["b1"], dtype=f32),
                "w2": w2_bf16,
                "b2": np.ascontiguousarray(inputs["b2"], dtype=f32),
                "v": np.ascontiguousarray(inputs["v"], dtype=f32),
            }
        )
    return in_maps


def run(inputs, trace=False):
    """Run on hardware. Returns (output [32, 1024] f32, exec_time_ns or None)."""
    nc = _get_nc()
    in_maps = make_in_maps(inputs)
    res = run_bass_kernel_spmd(
        nc, in_maps, core_ids=list(range(N_CORES)), trace=trace
    )
    out = np.concatenate([np.asarray(r["out"]) for r in res.results], axis=0)
    return out, res.exec_time_ns


def kernel(**inputs):
    out, _ = run(inputs)
    return out


# revision 103
# speedup vs baseline: 1.1582x; 1.0023x over previous
"""Bahdanau additive-attention kernel for one TRN2 chip (8 NeuronCores).

Reference computation (per batch b):
    q      = dec[b] @ w2 + b2 + b1                      # [1, E]
    H      = enc[b] @ w1                                # [S, E]
    scores = tanh(H + q) @ v (+ bv, softmax-invariant)  # [S, 1]
    attn   = softmax(scores over S)
    out[b] = attn @ enc[b]                              # [E]

Sharding: pure data-parallel over batch. 32 batches / 8 cores = 4 per core.
No collectives. Weights replicated. The host passes enc twice: transposed
([b, e, s]) in fp8-e4m3 for the H matmul, and natural layout in bf16 for the
context reduction.

The dominant H matmul runs in fp8 (e4m3) with MatmulPerfMode.DoubleRow: each
PE instruction contracts TWO 128-row k-chunks (lhsT [128,2,M], rhs [128,2,N])
at fp8's double rate - 2x the bf16/fp32r matmul throughput. w1 is pre-scaled
by 64 on the host so its [-1/32, 1/32] entries land in e4m3's normal range;
the 1/64 descale is fused into the ScalarE tanh (tanh(psum/64 + q)).
Quantization puts the end-to-end relative error at ~1.1e-2 (gate: 2e-2);
the fp8 products accumulate exactly in fp32 PSUM so hardware matches the
host-side estimate.

Per-core dataflow (B=4, S=2048, E=1024), working H^T = w1^T @ enc^T so the
tanh bias (q) is a per-partition scalar fused into the ScalarE activation:

  per s-block of 512:
    encT [e-chunk, s]   <- one consolidated DMA from host-transposed fp8 enc
    H^T chunks          <- 16 DoubleRow PE matmuls (w1 stationary)
    tanh(+q, /64)       <- ScalarE, PSUM -> SBUF (bf16)
    [lagged 1 block]  scores[1, s] = v^T @ tanh as 8 PE matmuls (vT column
                      stationary, tanh moving, PSUM-accumulated)
                      exp on ScalarE (+running sums); attn weights to DRAM
                      and back transposed ([s%128, s/128] layout)
    [lagged 2 blocks] ctx[1, E] += attn^T @ enc chunks (DVE, bf16 enc)
  softmax normalization is deferred to one final scale by 1/sum(exp):
  scores are bounded (|tanh|<1, v fixed) so no max-subtraction is needed.

The one-block lag of the v/exp stage and two-block lag of the context stage
keep the PE stream dense. The q projection (dec @ w2, computed directly in
transposed [e-part, b] layout) is injected into the PE stream after the
third H group so the opening matmuls never wait on the 2MB w2 load; the
first three tanhs are deferred until q lands. For the last batch the ctx
chain drains at lag 1 and its final s-block contributes via rank-1 PE
matmuls accumulated straight into the output-reduction PSUM group, keeping
the serial DVE chain out of the kernel's drain tail.

HW notes learned the hard way (all deterministic, simulator-invisible):
  - the first DMA into an SBUF region reused from earlier-scope tiles, when
    queued near 4-byte-stride gather descriptors, lands with the low 12
    mantissa bits of each aligned word zeroed -> main pools are allocated
    before the setup pool and the first encT tile is prefetched before any
    q-side DMAs;
  - fp8-typed ExternalInput uploads can corrupt; fp8 bytes travel as uint8
    and the DRAM APs are bitcast to fp8 in-kernel;
  - SBUF->SBUF partition-scatter DMA corrupts -> the exp transpose goes
    through DRAM;
  - each dma_start costs ~0.4us of sync-queue dispatch -> multi-chunk
    loads are consolidated into single multi-dim DMAs.
"""

import os
import sys

sys.path.insert(0, "/opt/trn_rl_repo")

import numpy as np  # noqa: E402

import concourse.tile as tile  # noqa: E402
from concourse import bacc, mybir  # noqa: E402
from concourse.bass import ts  # noqa: E402
from concourse.bass_utils import run_bass_kernel_spmd  # noqa: E402

P = 128
N_CORES = 8
B_TOTAL = 32
B = B_TOTAL // N_CORES  # 4 batches per core
S = 2048
E = 1024
EC = E // P  # 8 chunks of the hidden dim
EC2 = EC // 2  # 4 double-chunks (DoubleRow pairs)
SB = 512  # s-block (matmul moving size)
NSB = S // SB  # 4 s-blocks per batch
SK = S // P  # 16 s-chunks of 128 per batch
KSB = SB // P  # 4 s-chunks per s-block

F32 = mybir.dt.float32
F32R = mybir.dt.float32r
BF16 = mybir.dt.bfloat16
F8 = mybir.dt.float8e4  # e4m3
U8 = mybir.dt.uint8  # fp8 bytes travel as uint8: the fp8-typed host->device
# upload path corrupts part of the array; same bytes as uint8 arrive intact

W1_SCALE = 64.0  # host multiplies w1 by this before fp8 quantization

SD = F32R  # storage dtype of the DVE-side dataflow (bitcast f32)
Act = mybir.ActivationFunctionType
DR = mybir.MatmulPerfMode.DoubleRow

# bisection switches (temporary): set to "f32r" to revert a piece to baseline
Q_DT = F32R if os.environ.get("ATTN_Q") == "f32r" else BF16
CENC_DT = F32R if os.environ.get("ATTN_CENC") == "f32r" else BF16
H_FP8 = os.environ.get("ATTN_H") != "f32r"
WARM_GROUPS = int(os.environ.get("ATTN_WARM", "0"))


def _f32(ap):
    return ap if ap.dtype is F32 else ap.bitcast(F32)


DEBUG = os.environ.get("ATTN_DEBUG") == "1"


def _build_body(nc, tc, ctx, enc, encT_d, dec, w1, b1, w2, b2, v, out, dbg):
    # ---------------- persistent constants ----------------
    const = ctx.enter_context(tc.tile_pool(name="const", bufs=1))
    dram = ctx.enter_context(tc.tile_pool(name="dram", bufs=2, space="DRAM"))

    qT = const.tile([P, EC, B], F32)  # [p, c, b] = q_full[b, c*128+p]
    ones_f = const.tile([P, 1], F32)
    ones_sd = const.tile([P, 1], SD, name="ones_sd")
    ones_b = const.tile([P, 1], BF16, name="ones_b")
    nc.vector.memset(ones_f[:], 1.0)
    nc.vector.tensor_copy(ones_sd[:], ones_f[:])
    nc.vector.memset(ones_b[:], 1.0)

    # ---------------- main pools ----------------
    # Created BEFORE the setup pool: the first encT DMA must not land in a
    # region previously touched by the setup tiles / the 4-byte-stride qT
    # gather DMAs -- on HW that combination deterministically truncated the
    # low mantissa bits of the first encT tile (reduced-precision DMA path).
    encT_pool = ctx.enter_context(tc.tile_pool(name="encT", bufs=3))
    cenc_pool = ctx.enter_context(tc.tile_pool(name="cenc", bufs=5))
    work = ctx.enter_context(tc.tile_pool(name="work", bufs=18))
    accp = ctx.enter_context(tc.tile_pool(name="accp", bufs=2))
    onep = ctx.enter_context(tc.tile_pool(name="onep", bufs=2))
    ps_h = ctx.enter_context(tc.tile_pool(name="ps_h", bufs=4, space="PSUM"))
    ps_s = ctx.enter_context(tc.tile_pool(name="ps_s", bufs=1, space="PSUM"))
    ps_c = ctx.enter_context(tc.tile_pool(name="ps_c", bufs=1, space="PSUM"))

    def encT_dma(b, sb, split=1):
        encT = encT_pool.tile([P, EC, SB], F8 if H_FP8 else SD, tag="encT")
        encT_ap = encT_d[:].bitcast(F8) if H_FP8 else encT_d[:]
        encT_r = encT_ap[b].rearrange("(c p) s -> p c s", p=P)
        g = EC // split
        for i in range(split):
            nc.sync.dma_start(
                encT[:, ts(i, g), :], encT_r[:, ts(i, g), ts(sb, SB)]
            )
        return encT

    # ---- setup (pools stay open: the deferred q issue uses them later) ----
    if True:
        setup = ctx.enter_context(tc.tile_pool(name="setup", bufs=1))
        setup_ps = ctx.enter_context(
            tc.tile_pool(name="setup_ps", bufs=1, space="PSUM")
        )
        w2_sb = setup.tile([P, EC, E], Q_DT)
        w2_r = w2[:].rearrange("(c p) e -> p c e", p=P)
        # [p, b, c] layout so the gather balances into ONE dma_start
        # (the [p, c, b] variant needs 4 unbalanceable dims)
        decT = setup.tile([P, B, EC], Q_DT)  # [p, b, c] = dec[b, 0, c*128+p]
        dec_r = dec[:][:, 0, :].rearrange("b (c p) -> p b c", p=P)
        nc.sync.dma_start(decT[:], dec_r[:])
        b12T = setup.tile([P, EC], F32)
        b1_sb = setup.tile([P, EC], F32)
        b2_sb = setup.tile([P, EC], F32)
        nc.sync.dma_start(b1_sb[:], b1[:].rearrange("(c p) -> p c", p=P))
        nc.sync.dma_start(b2_sb[:], b2[:].rearrange("(c p) -> p c", p=P))
        nc.vector.tensor_add(b12T[:], b1_sb[:], b2_sb[:])

        # w1 + the first encT tile stream FIRST: they gate the opening H
        # matmuls. w2 (which only gates q/tanh) streams after them.
        # two halves: the opening DoubleRow instructions only need the first
        # k-chunk pairs, so they can start after half the w1 transfer
        w1_sb = const.tile([P, EC, E], F8 if H_FP8 else SD)  # w1[c*128+p, e']
        w1_ap = w1[:].bitcast(F8) if H_FP8 else w1[:]
        w1_r = w1_ap.rearrange("(c p) e -> p c e", p=P)
        for i in range(2):
            nc.sync.dma_start(w1_sb[:, ts(i, 4), :], w1_r[:, ts(i, 4), :])
        vT = const.tile([P, EC], SD)  # [p, c] = v[c*128+p, 0]
        nc.sync.dma_start(vT[:], v[:][:, 0].rearrange("(c p) -> p c", p=P))
        vT_b = const.tile([P, EC], BF16, name="vT_b")  # v-matmul stationary
        nc.vector.tensor_copy(vT_b[:], _f32(vT[:]))

        # prefetch the first s-block's encT ahead of the q/qT DMAs (see the
        # main-pool comment: ordering after them corrupts this tile on HW)
        encT_first = encT_dma(0, 0)
        nc.sync.dma_start(w2_sb[:], w2_r[:])

        # q is computed directly in [e'-partition, b] layout: stationary w2
        # chunk, moving decT columns -> PSUM [128, B]; bias add fuses b1+b2.
        # Deferred: issued into the PE stream AFTER the first H group so the
        # opening H matmuls don't wait behind the 2MB w2 load.
        def issue_q():
            for cp in range(EC):
                q_ps = setup_ps.tile([P, B], F32, tag="q_ps")
                for c in range(EC):
                    nc.tensor.matmul(
                        q_ps[:],
                        w2_sb[:, c, ts(cp, P)],
                        decT[:, :, c],
                        start=(c == 0),
                        stop=(c == EC - 1),
                    )
                nc.vector.tensor_scalar_add(
                    qT[:, cp, :], q_ps[:], b12T[:, cp : cp + 1]
                )
            if DEBUG:
                nc.sync.dma_start(dbg["qT"][:], qT[:])

    # PE warm-up: on HW the first fp8-DoubleRow window after the f32r/bf16
    # q matmuls computes corrupted PSUM (first-s-block-of-batch-0 signature;
    # later identical instructions are fine). Burn that window on dummy
    # DoubleRow groups whose results are discarded (a token column is DMA'd
    # out so the instructions aren't dead-code-eliminated).
    if H_FP8 and WARM_GROUPS > 0:
        warm_sb = const.tile([P, WARM_GROUPS], F32, name="warm_sb")
        for g in range(WARM_GROUPS):
            wps = ps_h.tile([P, SB], F32, tag="ph")
            for c2 in range(EC2):
                nc.tensor.matmul(
                    wps[:],
                    w1_sb[:, 2 * c2 : 2 * c2 + 2, 0:P],
                    w1_sb[:, 2 * c2 : 2 * c2 + 2, 0:SB],
                    start=(c2 == 0),
                    stop=(c2 == EC2 - 1),
                    perf_mode=DR,
                )
            nc.vector.tensor_copy(warm_sb[:, g : g + 1], wps[:, 0:1])
        warm_d = dram.tile([P, WARM_GROUPS], F32, tag="warm_d")
        nc.sync.dma_start(warm_d[:], warm_sb[:])

    # Work deferred so the PE never waits on ScalarE output or DMA
    # roundtrips: flushed one (v/exp) or two (ctx) s-blocks later.
    pending_v = []
    pending_ctx = []

    def flush_one(queue):
        if queue:
            queue.pop(0)()

    for b in range(B):
        a_dram = dram.tile([1, S], SD, tag="a_dram")
        sums = onep.tile([1, NSB], F32, tag="sums")
        expT = work.tile([P, SK], SD, tag="expT")  # [p, k] = exp[k*128+p]
        recip = onep.tile([1, 1], F32, tag="recip")
        cstate = {}  # running DVE accumulator for the context reduction

        for sb in range(NSB):
            # encT[p, c, j] = enc[b, sb*512+j, c*128+p], from host transpose.
            # The tile for block N+1 is DMA'd while block N computes (issue
            # pipelined one block ahead) so the PE never waits on it.
            if b == 0 and sb == 0:
                encT = encT_first
            else:
                encT = encT_prefetched  # noqa: F821 (set one iteration ago)
            nb, nsb = (b, sb + 1) if sb + 1 < NSB else (b + 1, 0)
            if nb < B:
                encT_prefetched = encT_dma(nb, nsb)
            # ---- main matmuls: H^T chunks via fp8 DoubleRow, tanh(+q) ----
            # Each DoubleRow instruction contracts e-chunks (2*c2, 2*c2+1):
            # lhsT [128, 2, 128] and rhs [128, 2, 512] pair along dim 1.
            def issue_tanh(ph, cp):
                th = work.tile([P, SB], BF16, tag="tanh")
                nc.scalar.activation(
                    th[:],
                    ph[:],
                    Act.Tanh,
                    bias=qT[:, cp, b : b + 1],
                    scale=(1.0 / W1_SCALE) if H_FP8 else 1.0,
                )
                return th

            ths = []
            pend_ph = []  # (b0, sb0): tanhs deferred until q lands in qT
            for cp in range(EC):
                ph = ps_h.tile([P, SB], F32, tag="ph")
                if H_FP8:
                    for c2 in range(EC2):
                        nc.tensor.matmul(
                            ph[:],
                            w1_sb[:, 2 * c2 : 2 * c2 + 2, ts(cp, P)],
                            encT[:, 2 * c2 : 2 * c2 + 2, :],
                            start=(c2 == 0),
                            stop=(c2 == EC2 - 1),
                            perf_mode=DR,
                        )
                else:
                    for c in range(EC):
                        nc.tensor.matmul(
                            ph[:],
                            w1_sb[:, c, ts(cp, P)],
                            encT[:, c, :],
                            start=(c == 0),
                            stop=(c == EC - 1),
                        )
                if b == 0 and sb == 0 and cp <= 2:
                    # hold the first tanhs: their qT bias is produced by the
                    # q matmuls injected after the third H group (so the
                    # opening H stream never waits on the w2 load)
                    pend_ph.append((cp, ph))
                    if cp == 2:
                        issue_q()
                        ths.extend(issue_tanh(p, c) for c, p in pend_ph)
                else:
                    ths.append(issue_tanh(ph, cp))

            if DEBUG and b == 0 and sb <= 1:
                nc.sync.dma_start(dbg["encT"][:][sb], encT[:])

            # prefetch the natural-layout bf16 enc chunks this block's
            # (2-block lagged) ctx reduction will need; issued after the
            # mains so they stay off the startup-critical DMA window
            enc_b = enc[:][b].rearrange("(k p) e -> p k e", p=P)
            cenc = cenc_pool.tile([P, KSB, E], CENC_DT, tag="cenc")
            nc.sync.dma_start(cenc[:], enc_b[:, ts(sb, KSB), :])
            cencs = [cenc[:, j, :] for j in range(KSB)]

            flush_one(pending_v)
            if len(pending_ctx) >= 2:
                flush_one(pending_ctx)
            if b == B - 1:
                # drain the ctx backlog to lag 1 during the last batch so
                # the remaining DVE chain segments overlap the final PE
                # blocks instead of serializing into the tail
                flush_one(pending_ctx)

            def make_v(
                b=b,
                sb=sb,
                ths=ths,
                sums=sums,
                expT=expT,
                recip=recip,
                a_dram=a_dram,
            ):
                def issue():
                    # scores[1, s] = sum_e v[e] * tanh[e, s] on the PE:
                    # vT chunk is a 1-column stationary (cheap ldweights),
                    # the tanh tiles stream as moving data; accumulate the
                    # 8 e-chunks in PSUM
                    pss = ps_s.tile([1, SB], F32, tag="pss", name="pss")
                    for cp in range(EC):
                        nc.tensor.matmul(
                            pss[:],
                            vT_b[:, cp : cp + 1],
                            ths[cp][:],
                            start=(cp == 0),
                            stop=(cp == EC - 1),
                        )
                    # exp + running sums (no max needed: |scores| <= 32)
                    exp_sb = onep.tile([1, SB], SD, tag="exp", name="exp_sb")
                    nc.scalar.activation(
                        exp_sb[:],
                        pss[:],
                        Act.Exp,
                        accum_out=sums[:, sb : sb + 1],
                    )
                    # transpose into expT[p, k] = exp[k*128+p] via a DRAM
                    # roundtrip (SBUF->SBUF partition-scatter DMA corrupts)
                    nc.sync.dma_start(a_dram[:, ts(sb, SB)], exp_sb[:])
                    nc.sync.dma_start(
                        expT[:, ts(sb, KSB)],
                        a_dram[:][0, ts(sb, SB)].rearrange(
                            "(k p) -> p k", p=P
                        ),
                    )
                    if DEBUG and sb == NSB - 1 and b <= 1:
                        nc.sync.dma_start(dbg["expT"][:][b], _f32(expT[:]))
                    if sb == NSB - 1:
                        # softmax denominator: must be issued AFTER the
                        # final sums write (Tile deps follow program order)
                        ssum = onep.tile([1, 1], F32, tag="ssum", name="ssum")
                        nc.vector.tensor_reduce(
                            ssum[:],
                            sums[:],
                            mybir.AxisListType.X,
                            mybir.AluOpType.add,
                        )
                        nc.vector.reciprocal(recip[:], ssum[:])

                return issue

            def make_ctx(
                b=b,
                sb=sb,
                cencs=cencs,
                expT=expT,
                cstate=cstate,
                recip=recip,
                last=(sb == NSB - 1),
            ):
                def issue():
                    # acc2[p, e] += enc[k*128+p, e] * attn[k*128+p]
                    # (VectorE); partition-sum via ones-matmul at the end.
                    # For the very last s-block of the last batch the serial
                    # DVE chain would be the kernel's drain tail - instead
                    # its 4 chunks go straight into the final PSUM group as
                    # rank-1 PE matmuls (attn column stationary).
                    pe_tail = b == B - 1 and last
                    if not pe_tail:
                        for j, k in enumerate(range(sb * KSB, (sb + 1) * KSB)):
                            nxt = accp.tile(
                                [P, E], SD, tag=f"cacc{k % 2}", name="cacc"
                            )
                            attn_k = _f32(expT[:, k : k + 1])
                            if k == 0:
                                nc.vector.tensor_scalar_mul(
                                    nxt[:], cencs[j], attn_k
                                )
                            else:
                                nc.vector.scalar_tensor_tensor(
                                    nxt[:],
                                    cencs[j],
                                    attn_k,
                                    cstate["acc"][:],
                                    mybir.AluOpType.mult,
                                    mybir.AluOpType.add,
                                )
                            cstate["acc"] = nxt
                    if last:
                        acc2 = cstate["acc"]
                        if pe_tail:
                            expT_b = onep.tile(
                                [P, KSB], BF16, tag="expT_b", name="expT_b"
                            )
                            nc.vector.tensor_copy(
                                expT_b[:], _f32(expT[:, ts(sb, KSB)])
                            )
                        for h in range(E // SB):
                            psc = ps_c.tile(
                                [1, SB], F32, tag=f"psc{h}", name="psc"
                            )
                            nc.tensor.matmul(
                                psc[:],
                                ones_sd[:],
                                acc2[:, ts(h, SB)],
                                start=True,
                                stop=not pe_tail,
                            )
                            if pe_tail:
                                for j in range(KSB):
                                    nc.tensor.matmul(
                                        psc[:],
                                        expT_b[:, j : j + 1],
                                        cencs[j][:, ts(h, SB)],
                                        start=False,
                                        stop=(j == KSB - 1),
                                    )
                            ctx_sb = onep.tile(
                                [1, SB], F32, tag="ctx", name="ctx_sb"
                            )
                            nc.scalar.activation(
                                ctx_sb[:], psc[:], Act.Copy, scale=recip[:]
                            )
                            nc.sync.dma_start(
                                out[:][b : b + 1, ts(h, SB)], ctx_sb[:]
                            )

                return issue

            pending_v.append(make_v())
            pending_ctx.append(make_ctx())

    while pending_v or pending_ctx:
        flush_one(pending_v)
        flush_one(pending_ctx)


def build_nc():
    nc = bacc.Bacc(
        "TRN2", target_bir_lowering=False, debug=False, num_devices=N_CORES
    )
    enc = nc.dram_tensor("encoder_outputs", [B, S, E], CENC_DT, kind="ExternalInput")
    encT_d = nc.dram_tensor(
        "encoder_outputs_t", [B, E, S], U8 if H_FP8 else SD, kind="ExternalInput"
    )
    dec = nc.dram_tensor("decoder_output", [B, 1, E], Q_DT, kind="ExternalInput")
    w1 = nc.dram_tensor("w1", [E, E], U8 if H_FP8 else SD, kind="ExternalInput")
    b1 = nc.dram_tensor("b1", [E], F32, kind="ExternalInput")
    w2 = nc.dram_tensor("w2", [E, E], Q_DT, kind="ExternalInput")
    b2 = nc.dram_tensor("b2", [E], F32, kind="ExternalInput")
    v = nc.dram_tensor("v", [E, 1], SD, kind="ExternalInput")
    out = nc.dram_tensor("out", [B, E], F32, kind="ExternalOutput")
    dbg = {}
    if DEBUG:
        dbg["qT"] = nc.dram_tensor("dbg_qT", [P, EC, B], F32, kind="ExternalOutput")
        dbg["th"] = nc.dram_tensor("dbg_th", [4, P, SB], F32, kind="ExternalOutput")
        dbg["ph"] = nc.dram_tensor("dbg_ph", [P, SB], F32, kind="ExternalOutput")
        dbg["expT"] = nc.dram_tensor("dbg_expT", [2, P, SK], F32, kind="ExternalOutput")
        dbg["encT"] = nc.dram_tensor(
            "dbg_encT", [2, P, EC, SB], F8 if H_FP8 else SD,
            kind="ExternalOutput"
        )

    from contextlib import ExitStack

    with tile.TileContext(nc) as tc:
        with ExitStack() as ctx:
            _build_body(nc, tc, ctx, enc, encT_d, dec, w1, b1, w2, b2, v, out, dbg)
    nc.compile()
    return nc


_NC_CACHE = None


def _get_nc():
    global _NC_CACHE
    if _NC_CACHE is None:
        _NC_CACHE = build_nc()
    return _NC_CACHE


def make_in_maps(inputs):
    """Host-side prep: shard over batch, quantize (fp8 transposed enc for the
    H matmul, bf16 natural enc for the context stage, fp8 w1 scaled by 64)."""
    f32 = np.float32
    q_np = mybir.dt.np(Q_DT)
    cenc_np = mybir.dt.np(CENC_DT)
    h_np = mybir.dt.np(F8) if H_FP8 else f32
    enc_all = np.asarray(inputs["encoder_outputs"], dtype=f32)
    enc_bf16 = np.ascontiguousarray(enc_all.astype(cenc_np))
    encT_f8 = np.ascontiguousarray(
        enc_all.astype(h_np).transpose(0, 2, 1)
    )
    dec_bf16 = np.asarray(inputs["decoder_output"], dtype=f32).astype(q_np)
    w1_f32 = np.asarray(inputs["w1"], dtype=f32)
    w1_f8 = (w1_f32 * f32(W1_SCALE)).astype(h_np) if H_FP8 else w1_f32
    if H_FP8:
        encT_f8 = encT_f8.view(np.uint8)
        w1_f8 = w1_f8.view(np.uint8)
    w2_bf16 = np.asarray(inputs["w2"], dtype=f32).astype(q_np)
    in_maps = []
    for i in range(N_CORES):
        sl = slice(i * B, (i + 1) * B)
        in_maps.append(
            {
                "encoder_outputs": np.ascontiguousarray(enc_bf16[sl]),
                "encoder_outputs_t": encT_f8[sl],
                "decoder_output": np.ascontiguousarray(dec_bf16[sl]),
                "w1": w1_f8,
                "b1": np.ascontiguousarray(inputs["b1"], dtype=f32),
                "w2": w2_bf16,
                "b2": np.ascontiguousarray(inputs["b2"], dtype=f32),
                "v": np.ascontiguousarray(inputs["v"], dtype=f32),
            }
        )
    return in_maps


def run(inputs, trace=False):
    """Run on hardware. Returns (output [32, 1024] f32, exec_time_ns or None)."""
    nc = _get_nc()
    in_maps = make_in_maps(inputs)
    res = run_bass_kernel_spmd(
        nc, in_maps, core_ids=list(range(N_CORES)), trace=trace
    )
    out = np.concatenate([np.asarray(r["out"]) for r in res.results], axis=0)
    return out, res.exec_time_ns


def kernel(**inputs):
    out, _ = run(inputs)
    return out


# revision 104
# speedup vs baseline: 1.1845x; 1.0227x over previous
"""Bahdanau additive-attention kernel for one TRN2 chip (8 NeuronCores).

Reference computation (per batch b):
    q      = dec[b] @ w2 + b2 + b1                      # [1, E]
    H      = enc[b] @ w1                                # [S, E]
    scores = tanh(H + q) @ v (+ bv, softmax-invariant)  # [S, 1]
    attn   = softmax(scores over S)
    out[b] = attn @ enc[b]                              # [E]

Sharding: pure data-parallel over batch. 32 batches / 8 cores = 4 per core.
No collectives. Weights replicated. The host passes enc twice: transposed
([b, e, s]) in fp8-e4m3 for the H matmul, and natural layout in bf16 for the
context reduction.

The dominant H matmul runs in fp8 (e4m3) with MatmulPerfMode.DoubleRow: each
PE instruction contracts TWO 128-row k-chunks (lhsT [128,2,M], rhs [128,2,N])
at fp8's double rate - 2x the bf16/fp32r matmul throughput. w1 is pre-scaled
by 64 on the host so its [-1/32, 1/32] entries land in e4m3's normal range;
the 1/64 descale is fused into the ScalarE tanh (tanh(psum/64 + q)).
Quantization puts the end-to-end relative error at ~1.1e-2 (gate: 2e-2);
the fp8 products accumulate exactly in fp32 PSUM so hardware matches the
host-side estimate.

Per-core dataflow (B=4, S=2048, E=1024), working H^T = w1^T @ enc^T so the
tanh bias (q) is a per-partition scalar fused into the ScalarE activation:

  per s-block of 512:
    encT [e-chunk, s]   <- one consolidated DMA from host-transposed fp8 enc
    H^T chunks          <- 16 DoubleRow PE matmuls (w1 stationary)
    tanh(+q, /64)       <- ScalarE, PSUM -> SBUF (bf16)
    [lagged 1 block]  scores[1, s] = v^T @ tanh as 8 PE matmuls (vT column
                      stationary, tanh moving, PSUM-accumulated)
                      exp on ScalarE (+running sums); attn weights to DRAM
                      and back transposed ([s%128, s/128] layout)
    [lagged 2 blocks] ctx[1, E] += attn^T @ enc chunks (DVE, bf16 enc)
  softmax normalization is deferred to one final scale by 1/sum(exp):
  scores are bounded (|tanh|<1, v fixed) so no max-subtraction is needed.

The one-block lag of the v/exp stage and two-block lag of the context stage
keep the PE stream dense. The q projection (dec @ w2, computed directly in
transposed [e-part, b] layout) is injected into the PE stream after the
third H group so the opening matmuls never wait on the 2MB w2 load; the
first three tanhs are deferred until q lands. For the last batch the ctx
chain drains at lag 1 and its final s-block contributes via rank-1 PE
matmuls accumulated straight into the output-reduction PSUM group, keeping
the serial DVE chain out of the kernel's drain tail.

HW notes learned the hard way (all deterministic, simulator-invisible):
  - the first DMA into an SBUF region reused from earlier-scope tiles, when
    queued near 4-byte-stride gather descriptors, lands with the low 12
    mantissa bits of each aligned word zeroed -> main pools are allocated
    before the setup pool and the first encT tile is prefetched before any
    q-side DMAs;
  - fp8-typed ExternalInput uploads can corrupt; fp8 bytes travel as uint8
    and the DRAM APs are bitcast to fp8 in-kernel;
  - SBUF->SBUF partition-scatter DMA corrupts -> the exp transpose goes
    through DRAM;
  - each dma_start costs ~0.4us of sync-queue dispatch -> multi-chunk
    loads are consolidated into single multi-dim DMAs.
"""

import os
import sys

sys.path.insert(0, "/opt/trn_rl_repo")

import numpy as np  # noqa: E402

import concourse.tile as tile  # noqa: E402
from concourse import bacc, mybir  # noqa: E402
from concourse.bass import ts  # noqa: E402
from concourse.bass_utils import run_bass_kernel_spmd  # noqa: E402

P = 128
N_CORES = 8
B_TOTAL = 32
B = B_TOTAL // N_CORES  # 4 batches per core
S = 2048
E = 1024
EC = E // P  # 8 chunks of the hidden dim
EC2 = EC // 2  # 4 double-chunks (DoubleRow pairs)
SB = 512  # s-block (matmul moving size)
NSB = S // SB  # 4 s-blocks per batch
SK = S // P  # 16 s-chunks of 128 per batch
KSB = SB // P  # 4 s-chunks per s-block

F32 = mybir.dt.float32
F32R = mybir.dt.float32r
BF16 = mybir.dt.bfloat16
F8 = mybir.dt.float8e4  # e4m3
U8 = mybir.dt.uint8  # fp8 bytes travel as uint8: the fp8-typed host->device
# upload path corrupts part of the array; same bytes as uint8 arrive intact

W1_SCALE = 64.0  # host multiplies w1 by this before fp8 quantization

SD = F32R  # storage dtype of the DVE-side dataflow (bitcast f32)
Act = mybir.ActivationFunctionType
DR = mybir.MatmulPerfMode.DoubleRow

# bisection switches (temporary): set to "f32r" to revert a piece to baseline
Q_DT = F32R if os.environ.get("ATTN_Q") == "f32r" else BF16
CENC_DT = F32R if os.environ.get("ATTN_CENC") == "f32r" else BF16
H_FP8 = os.environ.get("ATTN_H") != "f32r"
WARM_GROUPS = int(os.environ.get("ATTN_WARM", "0"))


def _f32(ap):
    return ap if ap.dtype is F32 else ap.bitcast(F32)


DEBUG = os.environ.get("ATTN_DEBUG") == "1"


def _build_body(nc, tc, ctx, enc, encT_d, dec, w1, b1, w2, b2, v, out, dbg):
    # ---------------- persistent constants ----------------
    const = ctx.enter_context(tc.tile_pool(name="const", bufs=1))
    dram = ctx.enter_context(tc.tile_pool(name="dram", bufs=2, space="DRAM"))

    qT = const.tile([P, EC, B], F32)  # [p, c, b] = q_full[b, c*128+p]
    ones_f = const.tile([P, 1], F32)
    ones_sd = const.tile([P, 1], SD, name="ones_sd")
    ones_b = const.tile([P, 1], BF16, name="ones_b")
    nc.vector.memset(ones_f[:], 1.0)
    nc.vector.tensor_copy(ones_sd[:], ones_f[:])
    nc.vector.memset(ones_b[:], 1.0)

    # ---------------- main pools ----------------
    # Created BEFORE the setup pool: the first encT DMA must not land in a
    # region previously touched by the setup tiles / the 4-byte-stride qT
    # gather DMAs -- on HW that combination deterministically truncated the
    # low mantissa bits of the first encT tile (reduced-precision DMA path).
    encT_pool = ctx.enter_context(tc.tile_pool(name="encT", bufs=3))
    cenc_pool = ctx.enter_context(tc.tile_pool(name="cenc", bufs=5))
    work = ctx.enter_context(tc.tile_pool(name="work", bufs=18))
    accp = ctx.enter_context(tc.tile_pool(name="accp", bufs=2))
    onep = ctx.enter_context(tc.tile_pool(name="onep", bufs=2))
    ps_h = ctx.enter_context(tc.tile_pool(name="ps_h", bufs=4, space="PSUM"))
    ps_s = ctx.enter_context(tc.tile_pool(name="ps_s", bufs=1, space="PSUM"))
    ps_c = ctx.enter_context(tc.tile_pool(name="ps_c", bufs=1, space="PSUM"))

    def encT_dma(b, sb, split=1):
        encT = encT_pool.tile([P, EC, SB], F8 if H_FP8 else SD, tag="encT")
        encT_ap = encT_d[:].bitcast(F8) if H_FP8 else encT_d[:]
        encT_r = encT_ap[b].rearrange("(c p) s -> p c s", p=P)
        g = EC // split
        for i in range(split):
            nc.sync.dma_start(
                encT[:, ts(i, g), :], encT_r[:, ts(i, g), ts(sb, SB)]
            )
        return encT

    # ---- setup (pools stay open: the deferred q issue uses them later) ----
    if True:
        setup = ctx.enter_context(tc.tile_pool(name="setup", bufs=1))
        setup_ps = ctx.enter_context(
            tc.tile_pool(name="setup_ps", bufs=1, space="PSUM")
        )
        w2_sb = setup.tile([P, EC, E], Q_DT)
        w2_r = w2[:].rearrange("(c p) e -> p c e", p=P)
        # [p, b, c] layout so the gather balances into ONE dma_start
        # (the [p, c, b] variant needs 4 unbalanceable dims)
        decT = setup.tile([P, B, EC], Q_DT)  # [p, b, c] = dec[b, 0, c*128+p]
        dec_r = dec[:][:, 0, :].rearrange("b (c p) -> p b c", p=P)
        nc.sync.dma_start(decT[:], dec_r[:])
        b12T = setup.tile([P, EC], F32)
        b1_sb = setup.tile([P, EC], F32)
        b2_sb = setup.tile([P, EC], F32)
        nc.sync.dma_start(b1_sb[:], b1[:].rearrange("(c p) -> p c", p=P))
        nc.sync.dma_start(b2_sb[:], b2[:].rearrange("(c p) -> p c", p=P))
        nc.vector.tensor_add(b12T[:], b1_sb[:], b2_sb[:])

        # w1 + the first encT tile stream FIRST: they gate the opening H
        # matmuls. w2 (which only gates q/tanh) streams after them.
        # two halves: the opening DoubleRow instructions only need the first
        # k-chunk pairs, so they can start after half the w1 transfer
        w1_sb = const.tile([P, EC, E], F8 if H_FP8 else SD)  # w1[c*128+p, e']
        w1_ap = w1[:].bitcast(F8) if H_FP8 else w1[:]
        w1_r = w1_ap.rearrange("(c p) e -> p c e", p=P)
        for i in range(2):
            nc.sync.dma_start(w1_sb[:, ts(i, 4), :], w1_r[:, ts(i, 4), :])
        vT = const.tile([P, EC], SD)  # [p, c] = v[c*128+p, 0]
        nc.sync.dma_start(vT[:], v[:][:, 0].rearrange("(c p) -> p c", p=P))
        vT_b = const.tile([P, EC], BF16, name="vT_b")  # v-matmul stationary
        nc.vector.tensor_copy(vT_b[:], _f32(vT[:]))

        # prefetch the first s-block's encT ahead of the q/qT DMAs (see the
        # main-pool comment: ordering after them corrupts this tile on HW)
        encT_first = encT_dma(0, 0)
        # two halves on two queues: the q matmuls (and through them the
        # first tanhs) gate on w2's full arrival
        for i in range(2):
            nc.sync.dma_start(w2_sb[:, ts(i, 4), :], w2_r[:, ts(i, 4), :])

        # q is computed directly in [e'-partition, b] layout: stationary w2
        # chunk, moving decT columns -> PSUM [128, B]; bias add fuses b1+b2.
        # Deferred: issued into the PE stream AFTER the first H group so the
        # opening H matmuls don't wait behind the 2MB w2 load.
        def issue_q():
            for cp in range(EC):
                q_ps = setup_ps.tile([P, B], F32, tag="q_ps")
                for c in range(EC):
                    nc.tensor.matmul(
                        q_ps[:],
                        w2_sb[:, c, ts(cp, P)],
                        decT[:, :, c],
                        start=(c == 0),
                        stop=(c == EC - 1),
                    )
                nc.vector.tensor_scalar_add(
                    qT[:, cp, :], q_ps[:], b12T[:, cp : cp + 1]
                )
            if DEBUG:
                nc.sync.dma_start(dbg["qT"][:], qT[:])

    # PE warm-up: on HW the first fp8-DoubleRow window after the f32r/bf16
    # q matmuls computes corrupted PSUM (first-s-block-of-batch-0 signature;
    # later identical instructions are fine). Burn that window on dummy
    # DoubleRow groups whose results are discarded (a token column is DMA'd
    # out so the instructions aren't dead-code-eliminated).
    if H_FP8 and WARM_GROUPS > 0:
        warm_sb = const.tile([P, WARM_GROUPS], F32, name="warm_sb")
        for g in range(WARM_GROUPS):
            wps = ps_h.tile([P, SB], F32, tag="ph")
            for c2 in range(EC2):
                nc.tensor.matmul(
                    wps[:],
                    w1_sb[:, 2 * c2 : 2 * c2 + 2, 0:P],
                    w1_sb[:, 2 * c2 : 2 * c2 + 2, 0:SB],
                    start=(c2 == 0),
                    stop=(c2 == EC2 - 1),
                    perf_mode=DR,
                )
            nc.vector.tensor_copy(warm_sb[:, g : g + 1], wps[:, 0:1])
        warm_d = dram.tile([P, WARM_GROUPS], F32, tag="warm_d")
        nc.sync.dma_start(warm_d[:], warm_sb[:])

    # Work deferred so the PE never waits on ScalarE output or DMA
    # roundtrips: flushed one (v/exp) or two (ctx) s-blocks later.
    pending_v = []
    pending_ctx = []

    def flush_one(queue):
        if queue:
            queue.pop(0)()

    for b in range(B):
        a_dram = dram.tile([1, S], SD, tag="a_dram")
        sums = onep.tile([1, NSB], F32, tag="sums")
        expT = work.tile([P, SK], SD, tag="expT")  # [p, k] = exp[k*128+p]
        recip = onep.tile([1, 1], F32, tag="recip")
        cstate = {}  # running DVE accumulator for the context reduction

        for sb in range(NSB):
            # encT[p, c, j] = enc[b, sb*512+j, c*128+p], from host transpose.
            # The tile for block N+1 is DMA'd while block N computes (issue
            # pipelined one block ahead) so the PE never waits on it.
            if b == 0 and sb == 0:
                encT = encT_first
            else:
                encT = encT_prefetched  # noqa: F821 (set one iteration ago)
            nb, nsb = (b, sb + 1) if sb + 1 < NSB else (b + 1, 0)
            if nb < B:
                encT_prefetched = encT_dma(nb, nsb)
            # ---- main matmuls: H^T chunks via fp8 DoubleRow, tanh(+q) ----
            # Each DoubleRow instruction contracts e-chunks (2*c2, 2*c2+1):
            # lhsT [128, 2, 128] and rhs [128, 2, 512] pair along dim 1.
            def issue_tanh(ph, cp):
                th = work.tile([P, SB], BF16, tag="tanh")
                nc.scalar.activation(
                    th[:],
                    ph[:],
                    Act.Tanh,
                    bias=qT[:, cp, b : b + 1],
                    scale=(1.0 / W1_SCALE) if H_FP8 else 1.0,
                )
                return th

            ths = []
            pend_ph = []  # (b0, sb0): tanhs deferred until q lands in qT
            for cp in range(EC):
                ph = ps_h.tile([P, SB], F32, tag="ph")
                if H_FP8:
                    for c2 in range(EC2):
                        nc.tensor.matmul(
                            ph[:],
                            w1_sb[:, 2 * c2 : 2 * c2 + 2, ts(cp, P)],
                            encT[:, 2 * c2 : 2 * c2 + 2, :],
                            start=(c2 == 0),
                            stop=(c2 == EC2 - 1),
                            perf_mode=DR,
                        )
                else:
                    for c in range(EC):
                        nc.tensor.matmul(
                            ph[:],
                            w1_sb[:, c, ts(cp, P)],
                            encT[:, c, :],
                            start=(c == 0),
                            stop=(c == EC - 1),
                        )
                if b == 0 and sb == 0 and cp <= 2:
                    # hold the first tanhs: their qT bias is produced by the
                    # q matmuls injected after the third H group (so the
                    # opening H stream never waits on the w2 load)
                    pend_ph.append((cp, ph))
                    if cp == 2:
                        issue_q()
                        ths.extend(issue_tanh(p, c) for c, p in pend_ph)
                else:
                    ths.append(issue_tanh(ph, cp))

            if DEBUG and b == 0 and sb <= 1:
                nc.sync.dma_start(dbg["encT"][:][sb], encT[:])

            # prefetch the natural-layout bf16 enc chunks this block's
            # (2-block lagged) ctx reduction will need; issued after the
            # mains so they stay off the startup-critical DMA window
            enc_b = enc[:][b].rearrange("(k p) e -> p k e", p=P)
            cenc = cenc_pool.tile([P, KSB, E], CENC_DT, tag="cenc")
            nc.sync.dma_start(cenc[:], enc_b[:, ts(sb, KSB), :])
            cencs = [cenc[:, j, :] for j in range(KSB)]

            flush_one(pending_v)
            if len(pending_ctx) >= 2:
                flush_one(pending_ctx)
            if b == B - 1:
                # drain the ctx backlog to lag 1 during the last batch so
                # the remaining DVE chain segments overlap the final PE
                # blocks instead of serializing into the tail
                flush_one(pending_ctx)

            def make_v(
                b=b,
                sb=sb,
                ths=ths,
                sums=sums,
                expT=expT,
                recip=recip,
                a_dram=a_dram,
            ):
                def issue():
                    # scores[1, s] = sum_e v[e] * tanh[e, s] on the PE:
                    # vT chunk is a 1-column stationary (cheap ldweights),
                    # the tanh tiles stream as moving data; accumulate the
                    # 8 e-chunks in PSUM
                    pss = ps_s.tile([1, SB], F32, tag="pss", name="pss")
                    for cp in range(EC):
                        nc.tensor.matmul(
                            pss[:],
                            vT_b[:, cp : cp + 1],
                            ths[cp][:],
                            start=(cp == 0),
                            stop=(cp == EC - 1),
                        )
                    # exp + running sums (no max needed: |scores| <= 32)
                    exp_sb = onep.tile([1, SB], SD, tag="exp", name="exp_sb")
                    nc.scalar.activation(
                        exp_sb[:],
                        pss[:],
                        Act.Exp,
                        accum_out=sums[:, sb : sb + 1],
                    )
                    # transpose into expT[p, k] = exp[k*128+p] via a DRAM
                    # roundtrip (SBUF->SBUF partition-scatter DMA corrupts)
                    nc.sync.dma_start(a_dram[:, ts(sb, SB)], exp_sb[:])
                    nc.sync.dma_start(
                        expT[:, ts(sb, KSB)],
                        a_dram[:][0, ts(sb, SB)].rearrange(
                            "(k p) -> p k", p=P
                        ),
                    )
                    if DEBUG and sb == NSB - 1 and b <= 1:
                        nc.sync.dma_start(dbg["expT"][:][b], _f32(expT[:]))
                    if sb == NSB - 1:
                        # softmax denominator: must be issued AFTER the
                        # final sums write (Tile deps follow program order)
                        ssum = onep.tile([1, 1], F32, tag="ssum", name="ssum")
                        nc.vector.tensor_reduce(
                            ssum[:],
                            sums[:],
                            mybir.AxisListType.X,
                            mybir.AluOpType.add,
                        )
                        nc.vector.reciprocal(recip[:], ssum[:])

                return issue

            def make_ctx(
                b=b,
                sb=sb,
                cencs=cencs,
                expT=expT,
                cstate=cstate,
                recip=recip,
                last=(sb == NSB - 1),
            ):
                def issue():
                    # acc2[p, e] += enc[k*128+p, e] * attn[k*128+p]
                    # (VectorE); partition-sum via ones-matmul at the end.
                    # For the very last s-block of the last batch the serial
                    # DVE chain would be the kernel's drain tail - instead
                    # its 4 chunks go straight into the final PSUM group as
                    # rank-1 PE matmuls (attn column stationary).
                    pe_tail = b == B - 1 and last
                    if not pe_tail:
                        for j, k in enumerate(range(sb * KSB, (sb + 1) * KSB)):
                            nxt = accp.tile(
                                [P, E], SD, tag=f"cacc{k % 2}", name="cacc"
                            )
                            attn_k = _f32(expT[:, k : k + 1])
                            if k == 0:
                                nc.vector.tensor_scalar_mul(
                                    nxt[:], cencs[j], attn_k
                                )
                            else:
                                nc.vector.scalar_tensor_tensor(
                                    nxt[:],
                                    cencs[j],
                                    attn_k,
                                    cstate["acc"][:],
                                    mybir.AluOpType.mult,
                                    mybir.AluOpType.add,
                                )
                            cstate["acc"] = nxt
                    if last:
                        acc2 = cstate["acc"]
                        if pe_tail:
                            expT_b = onep.tile(
                                [P, KSB], BF16, tag="expT_b", name="expT_b"
                            )
                            nc.vector.tensor_copy(
                                expT_b[:], _f32(expT[:, ts(sb, KSB)])
                            )
                        for h in range(E // SB):
                            psc = ps_c.tile(
                                [1, SB], F32, tag=f"psc{h}", name="psc"
                            )
                            nc.tensor.matmul(
                                psc[:],
                                ones_sd[:],
                                acc2[:, ts(h, SB)],
                                start=True,
                                stop=not pe_tail,
                            )
                            if pe_tail:
                                for j in range(KSB):
                                    nc.tensor.matmul(
                                        psc[:],
                                        expT_b[:, j : j + 1],
                                        cencs[j][:, ts(h, SB)],
                                        start=False,
                                        stop=(j == KSB - 1),
                                    )
                            ctx_sb = onep.tile(
                                [1, SB], F32, tag="ctx", name="ctx_sb"
                            )
                            nc.scalar.activation(
                                ctx_sb[:], psc[:], Act.Copy, scale=recip[:]
                            )
                            nc.sync.dma_start(
                                out[:][b : b + 1, ts(h, SB)], ctx_sb[:]
                            )

                return issue

            pending_v.append(make_v())
            pending_ctx.append(make_ctx())

    while pending_v or pending_ctx:
        flush_one(pending_v)
        flush_one(pending_ctx)


def build_nc():
    nc = bacc.Bacc(
        "TRN2", target_bir_lowering=False, debug=False, num_devices=N_CORES
    )
    enc = nc.dram_tensor("encoder_outputs", [B, S, E], CENC_DT, kind="ExternalInput")
    encT_d = nc.dram_tensor(
        "encoder_outputs_t", [B, E, S], U8 if H_FP8 else SD, kind="ExternalInput"
    )
    dec = nc.dram_tensor("decoder_output", [B, 1, E], Q_DT, kind="ExternalInput")
    w1 = nc.dram_tensor("w1", [E, E], U8 if H_FP8 else SD, kind="ExternalInput")
    b1 = nc.dram_tensor("b1", [E], F32, kind="ExternalInput")
    w2 = nc.dram_tensor("w2", [E, E], Q_DT, kind="ExternalInput")
    b2 = nc.dram_tensor("b2", [E], F32, kind="ExternalInput")
    v = nc.dram_tensor("v", [E, 1], SD, kind="ExternalInput")
    out = nc.dram_tensor("out", [B, E], F32, kind="ExternalOutput")
    dbg = {}
    if DEBUG:
        dbg["qT"] = nc.dram_tensor("dbg_qT", [P, EC, B], F32, kind="ExternalOutput")
        dbg["th"] = nc.dram_tensor("dbg_th", [4, P, SB], F32, kind="ExternalOutput")
        dbg["ph"] = nc.dram_tensor("dbg_ph", [P, SB], F32, kind="ExternalOutput")
        dbg["expT"] = nc.dram_tensor("dbg_expT", [2, P, SK], F32, kind="ExternalOutput")
        dbg["encT"] = nc.dram_tensor(
            "dbg_encT", [2, P, EC, SB], F8 if H_FP8 else SD,
            kind="ExternalOutput"
        )

    from contextlib import ExitStack

    with tile.TileContext(nc) as tc:
        with ExitStack() as ctx:
            _build_body(nc, tc, ctx, enc, encT_d, dec, w1, b1, w2, b2, v, out, dbg)
    nc.compile()
    return nc


_NC_CACHE = None


def _get_nc():
    global _NC_CACHE
    if _NC_CACHE is None:
        _NC_CACHE = build_nc()
    return _NC_CACHE


def make_in_maps(inputs):
    """Host-side prep: shard over batch, quantize (fp8 transposed enc for the
    H matmul, bf16 natural enc for the context stage, fp8 w1 scaled by 64)."""
    f32 = np.float32
    q_np = mybir.dt.np(Q_DT)
    cenc_np = mybir.dt.np(CENC_DT)
    h_np = mybir.dt.np(F8) if H_FP8 else f32
    enc_all = np.asarray(inputs["encoder_outputs"], dtype=f32)
    enc_bf16 = np.ascontiguousarray(enc_all.astype(cenc_np))
    encT_f8 = np.ascontiguousarray(
        enc_all.astype(h_np).transpose(0, 2, 1)
    )
    dec_bf16 = np.asarray(inputs["decoder_output"], dtype=f32).astype(q_np)
    w1_f32 = np.asarray(inputs["w1"], dtype=f32)
    w1_f8 = (w1_f32 * f32(W1_SCALE)).astype(h_np) if H_FP8 else w1_f32
    if H_FP8:
        encT_f8 = encT_f8.view(np.uint8)
        w1_f8 = w1_f8.view(np.uint8)
    w2_bf16 = np.asarray(inputs["w2"], dtype=f32).astype(q_np)
    in_maps = []
    for i in range(N_CORES):
        sl = slice(i * B, (i + 1) * B)
        in_maps.append(
            {
                "encoder_outputs": np.ascontiguousarray(enc_bf16[sl]),
                "encoder_outputs_t": encT_f8[sl],
                "decoder_output": np.ascontiguousarray(dec_bf16[sl]),
                "w1": w1_f8,
                "b1": np.ascontiguousarray(inputs["b1"], dtype=f32),
                "w2": w2_bf16,
                "b2": np.ascontiguousarray(inputs["b2"], dtype=f32),
                "v": np.ascontiguousarray(inputs["v"], dtype=f32),
            }
        )
    return in_maps


def run(inputs, trace=False):
    """Run on hardware. Returns (output [32, 1024] f32, exec_time_ns or None)."""
    nc = _get_nc()
    in_maps = make_in_maps(inputs)
    res = run_bass_kernel_spmd(
        nc, in_maps, core_ids=list(range(N_CORES)), trace=trace
    )
    out = np.concatenate([np.asarray(r["out"]) for r in res.results], axis=0)
    return out, res.exec_time_ns


def kernel(**inputs):
    out, _ = run(inputs)
    return out
